# revision 32
# baseline (speedup 1.0000x reference)
"""Trainium2 Bass kernel for GQA attention (32 q heads / 16 kv heads, head_dim
128, L=2048, D=4608) with RoPE, tanh softcap 50, causal mask, o_proj.

Strategy: tensor-parallel over heads across 8 NeuronCores. Core c computes
q-heads 4c..4c+3 and kv-heads 2c..2c+1 end-to-end (QKV projections, RoPE,
softcapped causal attention, and the partial o_proj against its 512 columns of
wo); the host sums the 8 partial [L, D] outputs.

v13 design notes (v3 baseline 536-556us, v7 ~527us, v10 ~506us, v13
measures 502.6us; the changes target the measured losses: ~53us of
HAM-throttled time from phase-2 PE starvation, 12us startup, 9us of
early-phase-1 DMA waits, and scalar-engine congestion):
  - phase-1 chunk 0 is k-outer: the 36 contraction chunks stream Q(4)+K(2)
    matmuls into 6 parallel psum banks so x/wq DMA is consumed strictly in
    k (load) order; startup issues 512/1024-col wq/x/wk slivers first, so
    the first matmul starts as soon as the runtime DMA path opens (~10.5us,
    all framework preamble) and never outruns the loads. Chunks 1-3 are
    h-outer (x fully prefetched by then) so the six rope-drain chains
    stagger - bunching them at the phase boundary blocked the phase-2 PSUM
    pool handover behind ~14us of gpsimd work. Each chunk's V sweep covers
    its rope drains; chunk 3's V is deferred into phase 2 as PE filler.
    (Deferring chunk-3 K as well was tried and REGRESSED: its rope chain
    lands in the ACT+Pool queues mid-wave-1, delaying exp and the causal
    masks that PV waits on.)
  - per-pair exp (was per-quad): tanh f32-psum -> fp16 tt, exp tt -> pt
    immediately, halving the score->prob latency the PV units wait on.
  - phase-2 filler eligibility is pair-based: PV/transpose units of group g
    become eligible 2 score-pairs after g's last pair (was: 2 whole groups),
    so qA drains sooner and o_proj units unblock a group earlier - this
    removes the queue-starvation hole at nq boundaries that re-throttled the
    HAM clock gate (observed 10-17us half-clock windows). +3 pairs was tried
    and measured slightly worse. The pump budget per pair is wave-scoped
    adaptive: remaining queued filler cost divided by the pairs left in the
    CURRENT wave (clamped to [900, 3200]) - a fixed 2200ns/pair drained each
    wave's o_proj supply before its last groups; dividing by pairs left in
    the whole phase under-pumped mid-wave (both measured worse).
  - phase-2 waves run in nq order [1, 2, 0, 3] so each wave's tanh/exp
    latency is covered by exp-independent filler: wave 1 by the deferred-V
    units, wave 2 by fresh o_proj(1), thin wave 0 by o_proj(2) whose
    surplus carries into wave 3 alongside o_proj(0); the tail is o_proj(3),
    which is dense PE work. ([1,2,3,0] was tried and regressed badly: wave
    3 at group 8 reuses wave 1's double-buffered attnT tiles while
    o_proj(1) still holds reads on them.)
  - the pump_guard flushes deferred-V only at group >= 8 (not immediately -
    an earlier -10 tag made the guard dump all 15.6us of V filler before
    the first score pair, starving wave 1), and o_proj at group <= i-5 (its
    true attnT double-buffer hazard bound; i-3 front-loaded each wave's
    filler and starved the next wave's tail). PV(s) for s=0,1 only needs
    score tiles through the second-to-last pair, so it becomes eligible a
    pair earlier than s=2,3. Together these spread the filler into the
    exp-latency windows where the PE would otherwise idle and re-throttle
    (v7 -> v10 was -22us).
  - PV normalize reads the pv psum directly (DVE recip + scalar-mul);
    the scalar-engine staging copy is gone (19us off the ACT queue that
    tanh/exp share, and ~0.5us off each PV unit's latency).
  - PSUM->SBUF drains on DVE (GPSIMD/Pool cannot access PSUM); the SBUF-only
    causal-mask multiplies run on the otherwise-idle Pool engine.
  - o_proj psum pool is double-buffered; output staged and stored as bf16
    (host sums the 8 partials in f32); the final output row stores per-512
    so the closing transfer is small.
"""

from collections import deque

import numpy as np
import ml_dtypes

import concourse.bass as bass
import concourse.mybir as mybir
import concourse.tile as tile
from concourse.masks import make_identity
from concourse import bacc

F32 = mybir.dt.float32
FP16 = mybir.dt.float16
BF16 = mybir.dt.bfloat16
BF16_NP = ml_dtypes.bfloat16
AF = mybir.ActivationFunctionType

N_HEADS = 32
N_KV = 16
HEAD_DIM = 128
ROPE_THETA = 10000.0
SOFTCAP = 50.0
SCALE = 1.0 / 12.0  # 1/sqrt(144)
L = 2048
D = 4608
N_CORES = 8
QH = N_HEADS // N_CORES        # 4 local q heads
KVH = N_KV // N_CORES          # 2 local kv heads
KC = D // 128                  # 36 contraction chunks
NQ = L // 512                  # 4 l-chunks of 512
LT = L // 128                  # 16 l-tiles of 128
QUART = 9 * 512                # x/wq quarter width (9 k-chunks)


def _emit(nc):
    # DRAM tensors in SBUF-image layout (see make_in_maps)
    xq_d = nc.dram_tensor("xq", [NQ, 4, 128, QUART], BF16, kind="ExternalInput")
    wqq_d = nc.dram_tensor("wqq", [4, 128, QUART], BF16, kind="ExternalInput")
    wkh_d = nc.dram_tensor("wkh", [2, 128, 18 * 256], BF16, kind="ExternalInput")
    wvh_d = nc.dram_tensor("wvh", [2, 128, 18 * 256], BF16, kind="ExternalInput")
    wo4_d = nc.dram_tensor("wo4", [QH, 128, D], BF16, kind="ExternalInput")
    cost_d = nc.dram_tensor("cost", [128, L], BF16, kind="ExternalInput")
    sint_d = nc.dram_tensor("sint", [128, L], BF16, kind="ExternalInput")
    masks_d = nc.dram_tensor("masks", [128, 128], BF16, kind="ExternalInput")
    out_d = nc.dram_tensor("out", [L, D], BF16, kind="ExternalOutput")

    with tile.TileContext(nc) as tc:
        with (
            tc.tile_pool(name="const", bufs=1) as const,
            tc.tile_pool(name="persist", bufs=1) as persist,
            tc.tile_pool(name="xpb", bufs=1) as xpb,    # x chunks 1,3; chunk 3 outlives phase 1
            tc.tile_pool(name="wvp", bufs=1) as wvp,    # wv weights; outlive phase 1
        ):
            QT = [persist.tile([128, L], BF16, tag=f"qt{h}", name=f"qt{h}") for h in range(QH)]
            KT = [persist.tile([128, L], BF16, tag=f"kt{g}", name=f"kt{g}") for g in range(KVH)]
            # V extended with a ones column per k-tile: [128, 16*129]
            VE = [persist.tile([128, LT * 129], BF16, tag=f"ve{g}", name=f"ve{g}") for g in range(KVH)]

            ident = const.tile([128, 128], BF16)
            cost = const.tile([128, L], BF16)
            sint = const.tile([128, L], BF16)
            maskt = const.tile([128, 128], BF16)
            wvs = wvp.tile([128, KC * 256], BF16, name="wvs")

            deferred = _phase1(nc, tc, ident, cost, sint, maskt, wvs, QT, KT, VE,
                               xq_d, wqq_d, wkh_d, wvh_d, cost_d, sint_d,
                               masks_d, xpb)
            _phase2(nc, tc, ident, maskt, QT, KT, VE, wvs, wo4_d, out_d, deferred)
    return nc


def _phase1(nc, tc, ident, cost, sint, maskt, wvs, QT, KT, VE,
            xq_d, wqq_d, wkh_d, wvh_d, cost_d, sint_d, masks_d, xpb):
    with (
        tc.tile_pool(name="xpa", bufs=1) as xpa,
        tc.tile_pool(name="wqk", bufs=1) as wqk,
        tc.tile_pool(name="rtmp", bufs=3) as rtmp,
        tc.tile_pool(name="qk_psum", bufs=1, space="PSUM") as qk_psum,
        tc.tile_pool(name="vp_psum", bufs=2, space="PSUM") as vp_psum,
    ):
        wqs = wqk.tile([128, KC * 512], BF16, name="wqs")
        wks = wqk.tile([128, KC * 256], BF16, name="wks")

        # startup: k=0 slivers first so the first matmuls start ~3us in;
        # the rest lands in k-ascending order, always ahead of the k-outer
        # consumption (one 512-col x slice feeds 6 N=512 matmuls = 1.3us,
        # vs ~0.4us to transfer).
        x_first = xpa.tile([128, KC * 512], BF16, tag="xa", name="x0")
        nc.sync.dma_start(wqs[:, 0:512], wqq_d[0][:, 0:512])
        nc.sync.dma_start(x_first[:, 0:512], xq_d[0, 0][:, 0:512])
        nc.sync.dma_start(wks[:, 0:256], wkh_d[0][:, 0:256])
        nc.sync.dma_start(wqs[:, 512:1536], wqq_d[0][:, 512:1536])
        nc.sync.dma_start(x_first[:, 512:1536], xq_d[0, 0][:, 512:1536])
        nc.sync.dma_start(wks[:, 256:768], wkh_d[0][:, 256:768])
        nc.sync.dma_start(wqs[:, 1536:2560], wqq_d[0][:, 1536:2560])
        nc.sync.dma_start(x_first[:, 1536:2560], xq_d[0, 0][:, 1536:2560])
        nc.sync.dma_start(wqs[:, 2560:QUART], wqq_d[0][:, 2560:QUART])
        nc.sync.dma_start(x_first[:, 2560:QUART], xq_d[0, 0][:, 2560:QUART])
        nc.sync.dma_start(wks[:, 768:18 * 256], wkh_d[0][:, 768:18 * 256])
        for j in range(1, 4):
            nc.sync.dma_start(wqs[:, j * QUART:(j + 1) * QUART], wqq_d[j])
            nc.sync.dma_start(x_first[:, j * QUART:(j + 1) * QUART], xq_d[0, j])
        nc.sync.dma_start(wks[:, 18 * 256:36 * 256], wkh_d[1])
        nc.sync.dma_start(cost[:], cost_d[:])
        nc.sync.dma_start(sint[:], sint_d[:])
        for j in range(2):
            nc.sync.dma_start(wvs[:, j * 18 * 256:(j + 1) * 18 * 256], wvh_d[j])
        nc.sync.dma_start(maskt[:], masks_d[:])
        make_identity(nc, ident[:])

        def load_x(nq, dst):
            for j in range(4):
                nc.sync.dma_start(dst[:, j * QUART:(j + 1) * QUART], xq_d[nq, j])

        def drain_rope(ps, dst, nq):
            """psum [128,512] f32 -> rope -> dst bf16 [128,512] slice."""
            cols = slice(nq * 512, (nq + 1) * 512)
            raw = rtmp.tile([128, 512], F32, tag="raw")
            nc.vector.tensor_mul(raw[:], ps[:], cost[:, cols])
            swap = rtmp.tile([128, 512], F32, tag="swap")
            nc.scalar.activation(swap[0:64, :], ps[64:128, :], AF.Copy)
            nc.scalar.activation(swap[64:128, :], ps[0:64, :], AF.Copy)
            nc.gpsimd.tensor_mul(swap[:], swap[:], sint[:, cols])
            nc.gpsimd.tensor_add(dst[:, cols], raw[:], swap[:])

        def alloc_x(nq):
            pool = xpb if nq % 2 == 1 else xpa
            tag = "xb" if nq % 2 == 1 else "xa"
            return pool.tile([128, KC * 512], BF16, tag=tag, name=f"x{nq}")

        xc = x_first
        for nq in range(NQ):
            if nq + 1 < NQ:
                xnext = alloc_x(nq + 1)
                load_x(nq + 1, xnext)
            if nq == 0:
                # k-outer: consumes x/wq strictly in k (DMA) order so the
                # startup never outruns the loads; drains bunch at the end
                # but overlap chunk 1's compute.
                psq = [qk_psum.tile([128, 512], F32, tag=f"q{h}", name=f"psq{h}")
                       for h in range(QH)]
                psk = [qk_psum.tile([128, 512], F32, tag=f"k{g}", name=f"psk{g}")
                       for g in range(KVH)]
                for k in range(KC):
                    for h in range(QH):
                        nc.tensor.matmul(
                            psq[h][:], wqs[:, k * 512 + h * 128:k * 512 + (h + 1) * 128],
                            xc[:, k * 512:(k + 1) * 512],
                            start=(k == 0), stop=(k == KC - 1))
                    for g in range(KVH):
                        nc.tensor.matmul(
                            psk[g][:], wks[:, k * 256 + g * 128:k * 256 + (g + 1) * 128],
                            xc[:, k * 512:(k + 1) * 512],
                            start=(k == 0), stop=(k == KC - 1))
                for h in range(QH):
                    drain_rope(psq[h], QT[h], nq)
                for g in range(KVH):
                    drain_rope(psk[g], KT[g], nq)
            else:
                # h-outer: x is fully prefetched by now; per-head psums drain
                # staggered so the engine queues (and the phase-2 handover for
                # chunk 3) never sit behind six bunched rope chains.
                for h in range(QH):
                    ps = qk_psum.tile([128, 512], F32, tag=f"q{h}", name=f"psq{h}")
                    for k in range(KC):
                        nc.tensor.matmul(
                            ps[:], wqs[:, k * 512 + h * 128:k * 512 + (h + 1) * 128],
                            xc[:, k * 512:(k + 1) * 512],
                            start=(k == 0), stop=(k == KC - 1))
                    drain_rope(ps, QT[h], nq)
                for g in range(KVH):
                    ps = qk_psum.tile([128, 512], F32, tag=f"k{g}", name=f"psk{g}")
                    for k in range(KC):
                        nc.tensor.matmul(
                            ps[:], wks[:, k * 256 + g * 128:k * 256 + (g + 1) * 128],
                            xc[:, k * 512:(k + 1) * 512],
                            start=(k == 0), stop=(k == KC - 1))
                    drain_rope(ps, KT[g], nq)
            if nq == NQ - 1:
                return [(nq, sub, xc) for sub in range(4)]  # V deferred to phase 2
            # V sweep of this chunk; its PE work covers the rope drains above
            for sub in range(4):
                mk = nq * 4 + sub
                ps = vp_psum.tile([128, KVH * 128], F32, tag="vps")
                for k in range(KC):
                    nc.tensor.matmul(
                        ps[:], xc[:, k * 512 + sub * 128:k * 512 + (sub + 1) * 128],
                        wvs[:, k * 256:(k + 1) * 256],
                        start=(k == 0), stop=(k == KC - 1))
                for g in range(KVH):
                    nc.vector.tensor_copy(
                        VE[g][:, mk * 129:mk * 129 + 128],
                        ps[:, g * 128:(g + 1) * 128])
                    nc.gpsimd.memset(VE[g][:, mk * 129 + 128:mk * 129 + 129], 1.0)
            xc = xnext


def _phase2(nc, tc, ident, maskt, QT, KT, VE, wvs, wo4_d, out_d, deferred_v):
    with (
        tc.tile_pool(name="wo", bufs=1) as wop,
        tc.tile_pool(name="pt", bufs=3) as ptp,
        tc.tile_pool(name="tt", bufs=2) as ttp,
        tc.tile_pool(name="attnt", bufs=2) as attp,
        tc.tile_pool(name="small", bufs=2) as small,
        tc.tile_pool(name="ostage", bufs=2) as ostage,
        tc.tile_pool(name="sc_psum", bufs=2, space="PSUM") as sc_psum,
        tc.tile_pool(name="pv_psum", bufs=1, space="PSUM") as pv_psum,
        tc.tile_pool(name="op_psum", bufs=2, space="PSUM") as op_psum,
        tc.tile_pool(name="atr_psum", bufs=1, space="PSUM") as atr_psum,
    ):
        WO = wop.tile([128, QH * D], BF16, name="wos")
        for h in range(QH):
            nc.sync.dma_start(WO[:, h * D:(h + 1) * D], wo4_d[h])

        # Wave order [1, 2, 0, 3]: each wave's softcap/exp latency is covered
        # by exp-independent PE filler - wave 1 by the deferred-V units (plus
        # its own larger score matmuls), wave 2 by fresh o_proj(1), thin wave
        # 0 by o_proj(2) whose surplus carries into wave 3 alongside
        # o_proj(0); the tail is o_proj(3), which is dense PE work. nq-major
        # order instead starved waves 2-3 (filler exhausted early) and
        # re-throttled the HAM clock gate.
        groups = [(nq, h) for nq in (1, 2, 0, 3) for h in range(QH)]
        state = {}
        att_of = {nq: [None] * QH for nq in range(NQ)}

        # Two filler queues:
        #   qA: exp-dependent units (PV / transpose), entries
        #       (elig_pair, group, pe_cost_ns, emit_fn) - eligible once the
        #       global score-pair counter passes 2 pairs beyond their group.
        #   qB: exp-independent units (o_proj, deferred V), entries
        #       (group, pe_cost_ns, emit_fn) - eligible once qA holds nothing
        #       at or before their group (preserves the T(nq,3) -> o_proj(nq)
        #       emission-order dependency).
        qA = deque()
        qB = deque()
        pairs_done = [0]
        prefer_b = [False]

        def emit_next(i):
            """Emit one eligible filler unit; return its PE cost or None.
            The two queues alternate so a PV unit's psum-recycle (DVE
            recip+mul) never sits behind several o_proj drain casts in the
            DVE queue. qB entries with group None (deferred V) have no
            ordering dependency on qA at all."""
            a_ok = qA and qA[0][0] <= pairs_done[0]
            b_ok = qB and (qB[0][0] is None or not qA or qA[0][1] > qB[0][0])
            use_b = b_ok and (prefer_b[0] or not a_ok)
            if use_b:
                g, cost, fn = qB.popleft()
                prefer_b[0] = False
            elif a_ok:
                e, g, cost, fn = qA.popleft()
                prefer_b[0] = True
            else:
                return None
            fn()
            return cost

        def pump(i):
            ns = 2200
            while ns > 0:
                c = emit_next(i)
                if c is None:
                    return
                ns -= c

        def pump_guard(i):
            # bound pipeline lag: PV/tr of group <= i-3 must be emitted
            # before group i reuses their pt buffer (pt pool is
            # triple-buffered). o_proj of wave W only conflicts with wave
            # W+2's attnT writes (attnt pool is double-buffered), so its
            # bound is i-5 - flushing it at i-3 front-loaded each wave's
            # filler and starved the next wave's tail. Deferred V units
            # (group None) are only forced once wave 3 approaches needing
            # VE; otherwise they interleave into wave 1's exp-latency
            # windows via pump().
            while qA and qA[0][1] <= i - 3:
                qA.popleft()[3]()
            while qB and ((qB[0][0] is None and i >= 8)
                          or (qB[0][0] is not None and qB[0][0] <= i - 5)):
                qB.popleft()[2]()

        def make_v_unit(vnq, sub, xc):
            def emit():
                mk = vnq * 4 + sub
                ps = op_psum.tile([128, 512], F32, tag="op")
                for k in range(KC):
                    nc.tensor.matmul(
                        ps[:, 0:KVH * 128],
                        xc[:, k * 512 + sub * 128:k * 512 + (sub + 1) * 128],
                        wvs[:, k * 256:(k + 1) * 256],
                        start=(k == 0), stop=(k == KC - 1))
                for g in range(KVH):
                    nc.vector.tensor_copy(
                        VE[g][:, mk * 129:mk * 129 + 128],
                        ps[:, g * 128:(g + 1) * 128])
                    nc.gpsimd.memset(VE[g][:, mk * 129 + 128:mk * 129 + 129], 1.0)
            return (None, 3900, emit)

        for u in deferred_v:
            qB.append(make_v_unit(*u))

        def emit_scores_pair(nq, h, pk, pt):
            """two score tiles into one [128,1024] f32 psum pair -> a single
            trimmed tanh into fp16 staging -> exp immediately (per-pair).
            Any stale psum between the two trimmed halves lands in pt columns
            that PV provably never reads."""
            g = h // 2
            ps = sc_psum.tile([128, 1024], F32, tag="sc")
            c0s = []
            for half in range(2):
                mk = 2 * pk + half
                o = mk - 4 * nq
                c0 = max(0, o) * 128
                c0s.append(c0)
                nc.tensor.matmul(
                    ps[:, half * 512 + c0:half * 512 + 512],
                    KT[g][:, mk * 128:(mk + 1) * 128],
                    QT[h][:, nq * 512 + c0:(nq + 1) * 512])
            tt = ttp.tile([128, 1024], FP16, tag="tt")
            nc.scalar.activation(
                tt[:, c0s[0]:1024], ps[:, c0s[0]:1024],
                AF.Tanh, scale=SCALE / SOFTCAP)
            nc.scalar.activation(
                pt[:, pk * 1024 + c0s[0]:(pk + 1) * 1024], tt[:, c0s[0]:1024],
                AF.Exp, scale=SOFTCAP)

        def emit_masks(nq, h, pt):
            # only the 128x128 block on the causal diagonal of each diagonal
            # tile is partially masked; everything left of it is never read
            for o in range(4):
                mk = 4 * nq + o
                base = mk * 512 + o * 128
                nc.gpsimd.tensor_mul(
                    pt[:, base:base + 128], pt[:, base:base + 128], maskt[:])

        def make_pv_unit(nq, h, s):
            def emit():
                st = state[(nq, h)]
                g = h // 2
                nks = 4 * nq + s + 1
                pv = pv_psum.tile([128, 129], F32, tag="pv")
                for mk in range(nks):
                    nc.tensor.matmul(
                        pv[:],
                        st["pt"][:, mk * 512 + s * 128:mk * 512 + (s + 1) * 128],
                        VE[g][:, mk * 129:(mk + 1) * 129],
                        start=(mk == 0), stop=(mk == nks - 1))
                recip = small.tile([128, 1], F32, tag="recip")
                nc.vector.reciprocal(recip[:], pv[:, 128:129])
                aq = small.tile([128, 128], BF16, tag=f"attnq{s}")
                nc.vector.tensor_scalar_mul(aq[:], pv[:, 0:128], recip[:])
                st["attnq"][s] = aq
            return 200 + (4 * nq + s + 1) * 110, emit

        def make_tr_unit(nq, h, s):
            def emit():
                st = state[(nq, h)]
                tp = atr_psum.tile([128, 128], BF16, tag="atr")
                nc.tensor.transpose(tp[:], st["attnq"][s][:], ident[:])
                nc.vector.tensor_copy(st["attnT"][:, s * 128:(s + 1) * 128], tp[:])
            return 180, emit

        def make_oproj_unit(idx, nq, s, j, ob, split_store=False):
            """single 512-wide o_proj chunk; DMA fires on each 3-chunk group
            boundary so at most one DVE drain sits between pumped PV units.
            The very last row stores per-512 so the final transfer is small."""
            def emit():
                po = op_psum.tile([128, 512], F32, tag="op")
                for h in range(QH):
                    nc.tensor.matmul(
                        po[:], att_of[nq][h][:, s * 128:(s + 1) * 128],
                        WO[:, h * D + j * 512:h * D + (j + 1) * 512],
                        start=(h == 0), stop=(h == QH - 1))
                nc.vector.tensor_copy(ob[:, (j % 3) * 512:(j % 3 + 1) * 512], po[:])
                row = nq * 512 + s * 128
                if split_store:
                    nc.sync.dma_start(
                        out_d[row:row + 128, j * 512:(j + 1) * 512],
                        ob[:, (j % 3) * 512:(j % 3 + 1) * 512])
                elif j % 3 == 2:
                    jg = j // 3
                    nc.sync.dma_start(
                        out_d[row:row + 128, jg * 1536:(jg + 1) * 1536], ob[:])
            return (idx, 1050, emit)

        for i, (nq, h) in enumerate(groups):
            npairs = 2 * nq + 2
            attnT = attp.tile([128, 512], BF16, tag=f"at{h}", name=f"at{h}")
            att_of[nq][h] = attnT
            pt = ptp.tile([128, LT * 512], BF16, tag="pt", name="pt")
            st = {"pt": pt, "attnq": [None] * 4, "attnT": attnT}
            state[(nq, h)] = st
            for pk in range(npairs):
                pump_guard(i)
                emit_scores_pair(nq, h, pk, pt)
                pairs_done[0] += 1
                pump(i)
            emit_masks(nq, h, pt)
            for s in range(4):
                # PV(s) only reads score tiles mk <= 4nq+s, so s=0,1 gate on
                # the second-to-last pair's exp and become eligible a pair
                # earlier than s=2,3.
                elig = pairs_done[0] + (1 if s < 2 else 2)
                cost, fn = make_pv_unit(nq, h, s)
                qA.append((elig, i, cost, fn))
                cost, fn = make_tr_unit(nq, h, s)
                qA.append((elig, i, cost, fn))
            if h == QH - 1:
                last_row = i == len(groups) - 1
                for s in range(4):
                    split = last_row and s == 3
                    ob = None
                    for j in range(D // 512):
                        if j % 3 == 0:
                            ob = ostage.tile([128, 1536], BF16, tag="ob")
                        qB.append(make_oproj_unit(i, nq, s, j, ob, split))
        while qA:
            qA.popleft()[3]()
        while qB:
            qB.popleft()[2]()


_CACHED_NC = {}


def build():
    if "nc" not in _CACHED_NC:
        nc = bacc.Bacc("TRN2", target_bir_lowering=False, debug=False)
        _emit(nc)
        nc.compile()
        _CACHED_NC["nc"] = nc
    return _CACHED_NC["nc"]


def host_tables():
    inv_freq = 1.0 / (ROPE_THETA ** (np.arange(0, HEAD_DIM, 2, dtype=np.float32) / HEAD_DIM))
    ang = np.arange(L, dtype=np.float32)[:, None] * inv_freq[None, :]  # [L, 64]
    cos, sin = np.cos(ang), np.sin(ang)
    cosT = np.concatenate([cos.T, cos.T], axis=0).astype(BF16_NP)
    sinT = np.concatenate([-sin.T, sin.T], axis=0).astype(BF16_NP)
    return np.ascontiguousarray(cosT), np.ascontiguousarray(sinT)


def host_masks():
    k = np.arange(128)[:, None]
    q = np.arange(128)[None, :]
    return np.ascontiguousarray((q >= k).astype(BF16_NP))  # [128, 128] triangle


def make_in_maps(x, wq, wk, wv, wo):
    cosT, sinT = host_tables()
    masks = host_masks()
    xt = x.reshape(L, D).T.astype(BF16_NP)  # [D, L]
    # x image: [nq, j, p, k9*512+c] = xt[(9j+k9)*128+p, nq*512+c]
    xi = xt.reshape(KC, 128, NQ, 512).transpose(2, 0, 1, 3)  # [NQ, KC, 128, 512]
    xi = xi.reshape(NQ, 4, 9, 128, 512).transpose(0, 1, 3, 2, 4)
    xi = np.ascontiguousarray(xi.reshape(NQ, 4, 128, QUART))
    in_maps = []
    for c in range(N_CORES):
        qs = slice(c * QH * 128, (c + 1) * QH * 128)
        kvs = slice(c * KVH * 128, (c + 1) * KVH * 128)
        wqt = wq[qs].T.astype(BF16_NP)   # [D, 512]
        wkt = wk[kvs].T.astype(BF16_NP)  # [D, 256]
        wvt = wv[kvs].T.astype(BF16_NP)
        wot = wo[:, qs].T.astype(BF16_NP)  # [512, D]
        wqi = wqt.reshape(KC, 128, 512).transpose(1, 0, 2)
        wqi = np.ascontiguousarray(
            wqi.reshape(128, 4, 9, 512).transpose(1, 0, 2, 3).reshape(4, 128, QUART))
        wki = wkt.reshape(KC, 128, 256).transpose(1, 0, 2)
        wki = np.ascontiguousarray(
            wki.reshape(128, 2, 18, 256).transpose(1, 0, 2, 3).reshape(2, 128, 18 * 256))
        wvi = wvt.reshape(KC, 128, 256).transpose(1, 0, 2)
        wvi = np.ascontiguousarray(
            wvi.reshape(128, 2, 18, 256).transpose(1, 0, 2, 3).reshape(2, 128, 18 * 256))
        wo4 = np.ascontiguousarray(wot.reshape(QH, 128, D))
        in_maps.append({
            "xq": xi,
            "wqq": wqi,
            "wkh": wki,
            "wvh": wvi,
            "wo4": wo4,
            "cost": cosT,
            "sint": sinT,
            "masks": masks,
        })
    return in_maps


def run(inputs, trace=False, trace_kwargs=None):
    from concourse.bass_utils import run_bass_kernel_spmd

    nc = build()
    x = np.asarray(inputs["x"], dtype=np.float32)
    in_maps = make_in_maps(
        x,
        np.asarray(inputs["wq"], dtype=np.float32),
        np.asarray(inputs["wk"], dtype=np.float32),
        np.asarray(inputs["wv"], dtype=np.float32),
        np.asarray(inputs["wo"], dtype=np.float32),
    )
    res = run_bass_kernel_spmd(
        nc, in_maps, core_ids=list(range(N_CORES)),
        trace=trace, **(trace_kwargs or {}))
    out = np.zeros((L, D), dtype=np.float32)
    for c in range(N_CORES):
        out += res.results[c]["out"].astype(np.float32)
    return out.reshape(x.shape), res


def kernel(**inputs) -> np.ndarray:
    out, _ = run(inputs, trace=False)
    return out


# revision 33
# speedup vs baseline: 1.1938x; 1.1938x over previous
"""Trainium2 Bass kernel for GQA attention (32 q heads / 16 kv heads, head_dim
128, L=2048, D=4608) with RoPE, tanh softcap 50, causal mask, o_proj.

Strategy: tensor-parallel over heads across 8 NeuronCores. Core c computes
q-heads 4c..4c+3 and kv-heads 2c..2c+1 end-to-end (QKV projections, RoPE,
softcapped causal attention, and the partial o_proj against its 512 columns of
wo); the host sums the 8 partial [L, D] outputs.

v13 design notes (v3 baseline 536-556us, v7 ~527us, v10 ~506us, v13
measures 502.6us; the changes target the measured losses: ~53us of
HAM-throttled time from phase-2 PE starvation, 12us startup, 9us of
early-phase-1 DMA waits, and scalar-engine congestion):
  - phase-1 chunk 0 is k-outer: the 36 contraction chunks stream Q(4)+K(2)
    matmuls into 6 parallel psum banks so x/wq DMA is consumed strictly in
    k (load) order; startup issues 512/1024-col wq/x/wk slivers first, so
    the first matmul starts as soon as the runtime DMA path opens (~10.5us,
    all framework preamble) and never outruns the loads. Chunks 1-3 are
    h-outer (x fully prefetched by then) so the six rope-drain chains
    stagger - bunching them at the phase boundary blocked the phase-2 PSUM
    pool handover behind ~14us of gpsimd work. Each chunk's V sweep covers
    its rope drains; chunk 3's V is deferred into phase 2 as PE filler.
    (Deferring chunk-3 K as well was tried and REGRESSED: its rope chain
    lands in the ACT+Pool queues mid-wave-1, delaying exp and the causal
    masks that PV waits on.)
  - per-pair exp (was per-quad): tanh f32-psum -> fp16 tt, exp tt -> pt
    immediately, halving the score->prob latency the PV units wait on.
  - phase-2 filler eligibility is pair-based: PV/transpose units of group g
    become eligible 2 score-pairs after g's last pair (was: 2 whole groups),
    so qA drains sooner and o_proj units unblock a group earlier - this
    removes the queue-starvation hole at nq boundaries that re-throttled the
    HAM clock gate (observed 10-17us half-clock windows). +3 pairs was tried
    and measured slightly worse. The pump budget per pair is wave-scoped
    adaptive: remaining queued filler cost divided by the pairs left in the
    CURRENT wave (clamped to [900, 3200]) - a fixed 2200ns/pair drained each
    wave's o_proj supply before its last groups; dividing by pairs left in
    the whole phase under-pumped mid-wave (both measured worse).
  - phase-2 waves run in nq order [1, 2, 0, 3] so each wave's tanh/exp
    latency is covered by exp-independent filler: wave 1 by the deferred-V
    units, wave 2 by fresh o_proj(1), thin wave 0 by o_proj(2) whose
    surplus carries into wave 3 alongside o_proj(0); the tail is o_proj(3),
    which is dense PE work. ([1,2,3,0] was tried and regressed badly: wave
    3 at group 8 reuses wave 1's double-buffered attnT tiles while
    o_proj(1) still holds reads on them.)
  - the pump_guard flushes deferred-V only at group >= 8 (not immediately -
    an earlier -10 tag made the guard dump all 15.6us of V filler before
    the first score pair, starving wave 1), and o_proj at group <= i-5 (its
    true attnT double-buffer hazard bound; i-3 front-loaded each wave's
    filler and starved the next wave's tail). PV(s) for s=0,1 only needs
    score tiles through the second-to-last pair, so it becomes eligible a
    pair earlier than s=2,3. Together these spread the filler into the
    exp-latency windows where the PE would otherwise idle and re-throttle
    (v7 -> v10 was -22us).
  - PV normalize reads the pv psum directly (DVE recip + scalar-mul);
    the scalar-engine staging copy is gone (19us off the ACT queue that
    tanh/exp share, and ~0.5us off each PV unit's latency).
  - PSUM->SBUF drains on DVE (GPSIMD/Pool cannot access PSUM); the SBUF-only
    causal-mask multiplies run on the otherwise-idle Pool engine.
  - o_proj psum pool is double-buffered; output staged and stored as bf16
    (host sums the 8 partials in f32); the final output row stores per-512
    so the closing transfer is small.
"""

from collections import deque

import numpy as np
import ml_dtypes

import concourse.bass as bass
import concourse.mybir as mybir
import concourse.tile as tile
from concourse.masks import make_identity
from concourse import bacc

F32 = mybir.dt.float32
FP16 = mybir.dt.float16
BF16 = mybir.dt.bfloat16
BF16_NP = ml_dtypes.bfloat16
AF = mybir.ActivationFunctionType

N_HEADS = 32
N_KV = 16
HEAD_DIM = 128
ROPE_THETA = 10000.0
SOFTCAP = 50.0
SCALE = 1.0 / 12.0  # 1/sqrt(144)
L = 2048
D = 4608
N_CORES = 8
QH = N_HEADS // N_CORES        # 4 local q heads
KVH = N_KV // N_CORES          # 2 local kv heads
KC = D // 128                  # 36 contraction chunks
NQ = L // 512                  # 4 l-chunks of 512
LT = L // 128                  # 16 l-tiles of 128
QUART = 9 * 512                # x/wq quarter width (9 k-chunks)


def _emit(nc):
    # DRAM tensors in SBUF-image layout (see make_in_maps)
    xq_d = nc.dram_tensor("xq", [NQ, 4, 128, QUART], BF16, kind="ExternalInput")
    wqq_d = nc.dram_tensor("wqq", [4, 128, QUART], BF16, kind="ExternalInput")
    wkh_d = nc.dram_tensor("wkh", [2, 128, 18 * 256], BF16, kind="ExternalInput")
    wvh_d = nc.dram_tensor("wvh", [2, 128, 18 * 256], BF16, kind="ExternalInput")
    wo4_d = nc.dram_tensor("wo4", [QH, 128, D], BF16, kind="ExternalInput")
    cost_d = nc.dram_tensor("cost", [128, L], BF16, kind="ExternalInput")
    sint_d = nc.dram_tensor("sint", [128, L], BF16, kind="ExternalInput")
    masks_d = nc.dram_tensor("masks", [128, 128], BF16, kind="ExternalInput")
    out_d = nc.dram_tensor("out", [L, D], BF16, kind="ExternalOutput")

    with tile.TileContext(nc) as tc:
        with (
            tc.tile_pool(name="const", bufs=1) as const,
            tc.tile_pool(name="persist", bufs=1) as persist,
            tc.tile_pool(name="xpb", bufs=1) as xpb,    # x chunks 1,3; chunk 3 outlives phase 1
            tc.tile_pool(name="wvp", bufs=1) as wvp,    # wv weights; outlive phase 1
        ):
            QT = [persist.tile([128, L], BF16, tag=f"qt{h}", name=f"qt{h}") for h in range(QH)]
            KT = [persist.tile([128, L], BF16, tag=f"kt{g}", name=f"kt{g}") for g in range(KVH)]
            # V extended with a ones column per k-tile: [128, 16*129]
            VE = [persist.tile([128, LT * 129], BF16, tag=f"ve{g}", name=f"ve{g}") for g in range(KVH)]

            ident = const.tile([128, 128], BF16)
            cost = const.tile([128, L], BF16)
            sint = const.tile([128, L], BF16)
            maskt = const.tile([128, 128], BF16)
            wvs = wvp.tile([128, KC * 256], BF16, name="wvs")

            deferred = _phase1(nc, tc, ident, cost, sint, maskt, wvs, QT, KT, VE,
                               xq_d, wqq_d, wkh_d, wvh_d, cost_d, sint_d,
                               masks_d, xpb)
            _phase2(nc, tc, ident, maskt, QT, KT, VE, wvs, wo4_d, out_d, deferred)
    return nc


def _phase1(nc, tc, ident, cost, sint, maskt, wvs, QT, KT, VE,
            xq_d, wqq_d, wkh_d, wvh_d, cost_d, sint_d, masks_d, xpb):
    with (
        tc.tile_pool(name="xpa", bufs=1) as xpa,
        tc.tile_pool(name="wqk", bufs=1) as wqk,
        tc.tile_pool(name="rtmp", bufs=3) as rtmp,
        tc.tile_pool(name="qk_psum", bufs=1, space="PSUM") as qk_psum,
        tc.tile_pool(name="vp_psum", bufs=2, space="PSUM") as vp_psum,
    ):
        wqs = wqk.tile([128, KC * 512], BF16, name="wqs")
        wks = wqk.tile([128, KC * 256], BF16, name="wks")

        # startup: k=0 slivers first so the first matmuls start ~3us in;
        # the rest lands in k-ascending order, always ahead of the k-outer
        # consumption (one 512-col x slice feeds 6 N=512 matmuls = 1.3us,
        # vs ~0.4us to transfer).
        x_first = xpa.tile([128, KC * 512], BF16, tag="xa", name="x0")
        nc.sync.dma_start(wqs[:, 0:512], wqq_d[0][:, 0:512])
        nc.sync.dma_start(x_first[:, 0:512], xq_d[0, 0][:, 0:512])
        nc.sync.dma_start(wks[:, 0:256], wkh_d[0][:, 0:256])
        nc.sync.dma_start(wqs[:, 512:1536], wqq_d[0][:, 512:1536])
        nc.sync.dma_start(x_first[:, 512:1536], xq_d[0, 0][:, 512:1536])
        nc.sync.dma_start(wks[:, 256:768], wkh_d[0][:, 256:768])
        nc.sync.dma_start(wqs[:, 1536:2560], wqq_d[0][:, 1536:2560])
        nc.sync.dma_start(x_first[:, 1536:2560], xq_d[0, 0][:, 1536:2560])
        nc.sync.dma_start(wqs[:, 2560:QUART], wqq_d[0][:, 2560:QUART])
        nc.sync.dma_start(x_first[:, 2560:QUART], xq_d[0, 0][:, 2560:QUART])
        nc.sync.dma_start(wks[:, 768:18 * 256], wkh_d[0][:, 768:18 * 256])
        for j in range(1, 4):
            nc.sync.dma_start(wqs[:, j * QUART:(j + 1) * QUART], wqq_d[j])
            nc.sync.dma_start(x_first[:, j * QUART:(j + 1) * QUART], xq_d[0, j])
        nc.sync.dma_start(wks[:, 18 * 256:36 * 256], wkh_d[1])
        nc.sync.dma_start(cost[:], cost_d[:])
        nc.sync.dma_start(sint[:], sint_d[:])
        for j in range(2):
            nc.sync.dma_start(wvs[:, j * 18 * 256:(j + 1) * 18 * 256], wvh_d[j])
        nc.sync.dma_start(maskt[:], masks_d[:])
        make_identity(nc, ident[:])

        def load_x(nq, dst):
            for j in range(4):
                nc.sync.dma_start(dst[:, j * QUART:(j + 1) * QUART], xq_d[nq, j])

        def drain_rope(ps, dst, nq):
            """psum [128,512] f32 -> rope -> dst bf16 [128,512] slice."""
            cols = slice(nq * 512, (nq + 1) * 512)
            raw = rtmp.tile([128, 512], F32, tag="raw")
            nc.vector.tensor_mul(raw[:], ps[:], cost[:, cols])
            swap = rtmp.tile([128, 512], F32, tag="swap")
            nc.scalar.activation(swap[0:64, :], ps[64:128, :], AF.Copy)
            nc.scalar.activation(swap[64:128, :], ps[0:64, :], AF.Copy)
            nc.gpsimd.tensor_mul(swap[:], swap[:], sint[:, cols])
            nc.gpsimd.tensor_add(dst[:, cols], raw[:], swap[:])

        def alloc_x(nq):
            pool = xpb if nq % 2 == 1 else xpa
            tag = "xb" if nq % 2 == 1 else "xa"
            return pool.tile([128, KC * 512], BF16, tag=tag, name=f"x{nq}")

        xc = x_first
        for nq in range(NQ):
            if nq + 1 < NQ:
                xnext = alloc_x(nq + 1)
                load_x(nq + 1, xnext)
            if nq == 0:
                # k-outer: consumes x/wq strictly in k (DMA) order so the
                # startup never outruns the loads; drains bunch at the end
                # but overlap chunk 1's compute.
                psq = [qk_psum.tile([128, 512], F32, tag=f"q{h}", name=f"psq{h}")
                       for h in range(QH)]
                psk = [qk_psum.tile([128, 512], F32, tag=f"k{g}", name=f"psk{g}")
                       for g in range(KVH)]
                for k in range(KC):
                    for h in range(QH):
                        nc.tensor.matmul(
                            psq[h][:], wqs[:, k * 512 + h * 128:k * 512 + (h + 1) * 128],
                            xc[:, k * 512:(k + 1) * 512],
                            start=(k == 0), stop=(k == KC - 1))
                    for g in range(KVH):
                        nc.tensor.matmul(
                            psk[g][:], wks[:, k * 256 + g * 128:k * 256 + (g + 1) * 128],
                            xc[:, k * 512:(k + 1) * 512],
                            start=(k == 0), stop=(k == KC - 1))
                for h in range(QH):
                    drain_rope(psq[h], QT[h], nq)
                for g in range(KVH):
                    drain_rope(psk[g], KT[g], nq)
            else:
                # h-outer: x is fully prefetched by now; per-head psums drain
                # staggered so the engine queues (and the phase-2 handover for
                # chunk 3) never sit behind six bunched rope chains.
                for h in range(QH):
                    ps = qk_psum.tile([128, 512], F32, tag=f"q{h}", name=f"psq{h}")
                    for k in range(KC):
                        nc.tensor.matmul(
                            ps[:], wqs[:, k * 512 + h * 128:k * 512 + (h + 1) * 128],
                            xc[:, k * 512:(k + 1) * 512],
                            start=(k == 0), stop=(k == KC - 1))
                    drain_rope(ps, QT[h], nq)
                for g in range(KVH):
                    ps = qk_psum.tile([128, 512], F32, tag=f"k{g}", name=f"psk{g}")
                    for k in range(KC):
                        nc.tensor.matmul(
                            ps[:], wks[:, k * 256 + g * 128:k * 256 + (g + 1) * 128],
                            xc[:, k * 512:(k + 1) * 512],
                            start=(k == 0), stop=(k == KC - 1))
                    drain_rope(ps, KT[g], nq)
            if nq == NQ - 1:
                return [(nq, sub, xc) for sub in range(4)]  # V deferred to phase 2
            # V sweep of this chunk; its PE work covers the rope drains above
            for sub in range(4):
                mk = nq * 4 + sub
                ps = vp_psum.tile([128, KVH * 128], F32, tag="vps")
                for k in range(KC):
                    nc.tensor.matmul(
                        ps[:], xc[:, k * 512 + sub * 128:k * 512 + (sub + 1) * 128],
                        wvs[:, k * 256:(k + 1) * 256],
                        start=(k == 0), stop=(k == KC - 1))
                for g in range(KVH):
                    nc.vector.tensor_copy(
                        VE[g][:, mk * 129:mk * 129 + 128],
                        ps[:, g * 128:(g + 1) * 128])
                    nc.gpsimd.memset(VE[g][:, mk * 129 + 128:mk * 129 + 129], 1.0)
            xc = xnext


def _phase2(nc, tc, ident, maskt, QT, KT, VE, wvs, wo4_d, out_d, deferred_v):
    with (
        tc.tile_pool(name="wo", bufs=1) as wop,
        tc.tile_pool(name="pt", bufs=3) as ptp,
        tc.tile_pool(name="tt", bufs=2) as ttp,
        tc.tile_pool(name="attnt", bufs=2) as attp,
        tc.tile_pool(name="small", bufs=2) as small,
        tc.tile_pool(name="ostage", bufs=2) as ostage,
        tc.tile_pool(name="sc_psum", bufs=2, space="PSUM") as sc_psum,
        tc.tile_pool(name="pv_psum", bufs=1, space="PSUM") as pv_psum,
        tc.tile_pool(name="op_psum", bufs=2, space="PSUM") as op_psum,
        tc.tile_pool(name="atr_psum", bufs=1, space="PSUM") as atr_psum,
    ):
        WO = wop.tile([128, QH * D], BF16, name="wos")
        for h in range(QH):
            nc.sync.dma_start(WO[:, h * D:(h + 1) * D], wo4_d[h])

        # Wave order [1, 2, 0, 3]: each wave's softcap/exp latency is covered
        # by exp-independent PE filler - wave 1 by the deferred-V units (plus
        # its own larger score matmuls), wave 2 by fresh o_proj(1), thin wave
        # 0 by o_proj(2) whose surplus carries into wave 3 alongside
        # o_proj(0); the tail is o_proj(3), which is dense PE work. nq-major
        # order instead starved waves 2-3 (filler exhausted early) and
        # re-throttled the HAM clock gate.
        groups = [(nq, h) for nq in (1, 2, 0, 3) for h in range(QH)]
        state = {}
        att_of = {nq: [None] * QH for nq in range(NQ)}

        # Two filler queues:
        #   qA: exp-dependent units (PV / transpose), entries
        #       (elig_pair, group, pe_cost_ns, emit_fn) - eligible once the
        #       global score-pair counter passes 2 pairs beyond their group.
        #   qB: exp-independent units (o_proj, deferred V), entries
        #       (group, pe_cost_ns, emit_fn) - eligible once qA holds nothing
        #       at or before their group (preserves the T(nq,3) -> o_proj(nq)
        #       emission-order dependency).
        qA = deque()
        qB = deque()
        pairs_done = [0]
        prefer_b = [False]
        queue_cost = [0]          # summed pe_cost_ns of everything queued

        def emit_next(i):
            """Emit one eligible filler unit; return its PE cost or None.
            The two queues alternate so a PV unit's psum-recycle (DVE
            recip+mul) never sits behind several o_proj drain casts in the
            DVE queue. qB entries with group None (deferred V) have no
            ordering dependency on qA at all."""
            a_ok = qA and qA[0][0] <= pairs_done[0]
            b_ok = qB and (qB[0][0] is None or not qA or qA[0][1] > qB[0][0])
            use_b = b_ok and (prefer_b[0] or not a_ok)
            if use_b:
                g, cost, fn = qB.popleft()
                prefer_b[0] = False
            elif a_ok:
                e, g, cost, fn = qA.popleft()
                prefer_b[0] = True
            else:
                return None
            fn()
            queue_cost[0] -= cost
            return cost

        def pump(i, pairs_left_in_wave):
            # spread the queued filler across the rest of this wave's score
            # pairs (+2 reserves a couple of units for the final flush): a
            # fixed 2200ns/pair drained each wave's o_proj supply before its
            # last groups, leaving them exp-gated in 1-3us clustered gaps
            # that re-throttle the HAM clock gate; spreading converts those
            # into harmless sub-window micro-gaps.
            ns = min(3200, max(900, queue_cost[0] // (pairs_left_in_wave + 2)))
            while ns > 0:
                c = emit_next(i)
                if c is None:
                    return
                ns -= c

        def pump_guard(i):
            # bound pipeline lag: PV/tr of group <= i-3 must be emitted
            # before group i reuses their pt buffer (pt pool is
            # triple-buffered). o_proj of wave W only conflicts with wave
            # W+2's attnT writes (attnt pool is double-buffered), so its
            # bound is i-5 - flushing it at i-3 front-loaded each wave's
            # filler and starved the next wave's tail. Deferred V units
            # (group None) are only forced once wave 3 approaches needing
            # VE; otherwise they interleave into wave 1's exp-latency
            # windows via pump().
            while qA and qA[0][1] <= i - 3:
                qA.popleft()[3]()
            while qB and ((qB[0][0] is None and i >= 8)
                          or (qB[0][0] is not None and qB[0][0] <= i - 5)):
                qB.popleft()[2]()

        def make_v_unit(vnq, sub, xc):
            def emit():
                mk = vnq * 4 + sub
                ps = op_psum.tile([128, 512], F32, tag="op")
                for k in range(KC):
                    nc.tensor.matmul(
                        ps[:, 0:KVH * 128],
                        xc[:, k * 512 + sub * 128:k * 512 + (sub + 1) * 128],
                        wvs[:, k * 256:(k + 1) * 256],
                        start=(k == 0), stop=(k == KC - 1))
                for g in range(KVH):
                    nc.vector.tensor_copy(
                        VE[g][:, mk * 129:mk * 129 + 128],
                        ps[:, g * 128:(g + 1) * 128])
                    nc.gpsimd.memset(VE[g][:, mk * 129 + 128:mk * 129 + 129], 1.0)
            return (None, 3900, emit)

        for u in deferred_v:
            unit = make_v_unit(*u)
            queue_cost[0] += unit[1]
            qB.append(unit)

        def emit_scores_pair(nq, h, pk, pt):
            """two score tiles into one [128,1024] f32 psum pair -> a single
            trimmed tanh into fp16 staging -> exp immediately (per-pair).
            Any stale psum between the two trimmed halves lands in pt columns
            that PV provably never reads."""
            g = h // 2
            ps = sc_psum.tile([128, 1024], F32, tag="sc")
            c0s = []
            for half in range(2):
                mk = 2 * pk + half
                o = mk - 4 * nq
                c0 = max(0, o) * 128
                c0s.append(c0)
                nc.tensor.matmul(
                    ps[:, half * 512 + c0:half * 512 + 512],
                    KT[g][:, mk * 128:(mk + 1) * 128],
                    QT[h][:, nq * 512 + c0:(nq + 1) * 512])
            tt = ttp.tile([128, 1024], FP16, tag="tt")
            nc.scalar.activation(
                tt[:, c0s[0]:1024], ps[:, c0s[0]:1024],
                AF.Tanh, scale=SCALE / SOFTCAP)
            nc.scalar.activation(
                pt[:, pk * 1024 + c0s[0]:(pk + 1) * 1024], tt[:, c0s[0]:1024],
                AF.Exp, scale=SOFTCAP)

        def emit_masks(nq, h, pt):
            # only the 128x128 block on the causal diagonal of each diagonal
            # tile is partially masked; everything left of it is never read
            for o in range(4):
                mk = 4 * nq + o
                base = mk * 512 + o * 128
                nc.gpsimd.tensor_mul(
                    pt[:, base:base + 128], pt[:, base:base + 128], maskt[:])

        def make_pv_unit(nq, h, s):
            def emit():
                st = state[(nq, h)]
                g = h // 2
                nks = 4 * nq + s + 1
                pv = pv_psum.tile([128, 129], F32, tag="pv")
                for mk in range(nks):
                    nc.tensor.matmul(
                        pv[:],
                        st["pt"][:, mk * 512 + s * 128:mk * 512 + (s + 1) * 128],
                        VE[g][:, mk * 129:(mk + 1) * 129],
                        start=(mk == 0), stop=(mk == nks - 1))
                recip = small.tile([128, 1], F32, tag="recip")
                nc.vector.reciprocal(recip[:], pv[:, 128:129])
                aq = small.tile([128, 128], BF16, tag=f"attnq{s}")
                nc.vector.tensor_scalar_mul(aq[:], pv[:, 0:128], recip[:])
                st["attnq"][s] = aq
            return 200 + (4 * nq + s + 1) * 110, emit

        def make_tr_unit(nq, h, s):
            def emit():
                st = state[(nq, h)]
                tp = atr_psum.tile([128, 128], BF16, tag="atr")
                nc.tensor.transpose(tp[:], st["attnq"][s][:], ident[:])
                nc.vector.tensor_copy(st["attnT"][:, s * 128:(s + 1) * 128], tp[:])
            return 180, emit

        def make_oproj_unit(idx, nq, s, j, ob, split_store=False):
            """single 512-wide o_proj chunk; DMA fires on each 3-chunk group
            boundary so at most one DVE drain sits between pumped PV units.
            The very last row stores per-512 so the final transfer is small."""
            def emit():
                po = op_psum.tile([128, 512], F32, tag="op")
                for h in range(QH):
                    nc.tensor.matmul(
                        po[:], att_of[nq][h][:, s * 128:(s + 1) * 128],
                        WO[:, h * D + j * 512:h * D + (j + 1) * 512],
                        start=(h == 0), stop=(h == QH - 1))
                nc.vector.tensor_copy(ob[:, (j % 3) * 512:(j % 3 + 1) * 512], po[:])
                row = nq * 512 + s * 128
                if split_store:
                    nc.sync.dma_start(
                        out_d[row:row + 128, j * 512:(j + 1) * 512],
                        ob[:, (j % 3) * 512:(j % 3 + 1) * 512])
                elif j % 3 == 2:
                    jg = j // 3
                    nc.sync.dma_start(
                        out_d[row:row + 128, jg * 1536:(jg + 1) * 1536], ob[:])
            return (idx, 1050, emit)

        for i, (nq, h) in enumerate(groups):
            npairs = 2 * nq + 2
            attnT = attp.tile([128, 512], BF16, tag=f"at{h}", name=f"at{h}")
            att_of[nq][h] = attnT
            pt = ptp.tile([128, LT * 512], BF16, tag="pt", name="pt")
            st = {"pt": pt, "attnq": [None] * 4, "attnT": attnT}
            state[(nq, h)] = st
            for pk in range(npairs):
                pump_guard(i)
                emit_scores_pair(nq, h, pk, pt)
                pairs_done[0] += 1
                pump(i, (4 - i % 4) * npairs - pk - 1)
            emit_masks(nq, h, pt)
            for s in range(4):
                # PV(s) only reads score tiles mk <= 4nq+s, so s=0,1 gate on
                # the second-to-last pair's exp and become eligible a pair
                # earlier than s=2,3.
                elig = pairs_done[0] + (1 if s < 2 else 2)
                cost, fn = make_pv_unit(nq, h, s)
                qA.append((elig, i, cost, fn))
                queue_cost[0] += cost
                cost, fn = make_tr_unit(nq, h, s)
                qA.append((elig, i, cost, fn))
                queue_cost[0] += cost
            if h == QH - 1:
                last_row = i == len(groups) - 1
                for s in range(4):
                    split = last_row and s == 3
                    ob = None
                    for j in range(D // 512):
                        if j % 3 == 0:
                            ob = ostage.tile([128, 1536], BF16, tag="ob")
                        unit = make_oproj_unit(i, nq, s, j, ob, split)
                        queue_cost[0] += unit[1]
                        qB.append(unit)
        # cover the last group's exp latency with reserved o_proj remnants
        # from earlier waves before its PV units hit the PE queue (never the
        # last wave's own o_proj - those must follow the qA transposes below)
        last_safe = len(groups) - 5
        for _ in range(3):
            if qB and qB[0][0] is not None and qB[0][0] <= last_safe:
                qB.popleft()[2]()
        while qA:
            qA.popleft()[3]()
        while qB:
            qB.popleft()[2]()


_CACHED_NC = {}


def build():
    if "nc" not in _CACHED_NC:
        nc = bacc.Bacc("TRN2", target_bir_lowering=False, debug=False)
        _emit(nc)
        nc.compile()
        _CACHED_NC["nc"] = nc
    return _CACHED_NC["nc"]


def host_tables():
    inv_freq = 1.0 / (ROPE_THETA ** (np.arange(0, HEAD_DIM, 2, dtype=np.float32) / HEAD_DIM))
    ang = np.arange(L, dtype=np.float32)[:, None] * inv_freq[None, :]  # [L, 64]
    cos, sin = np.cos(ang), np.sin(ang)
    cosT = np.concatenate([cos.T, cos.T], axis=0).astype(BF16_NP)
    sinT = np.concatenate([-sin.T, sin.T], axis=0).astype(BF16_NP)
    return np.ascontiguousarray(cosT), np.ascontiguousarray(sinT)


def host_masks():
    k = np.arange(128)[:, None]
    q = np.arange(128)[None, :]
    return np.ascontiguousarray((q >= k).astype(BF16_NP))  # [128, 128] triangle


def make_in_maps(x, wq, wk, wv, wo):
    cosT, sinT = host_tables()
    masks = host_masks()
    xt = x.reshape(L, D).T.astype(BF16_NP)  # [D, L]
    # x image: [nq, j, p, k9*512+c] = xt[(9j+k9)*128+p, nq*512+c]
    xi = xt.reshape(KC, 128, NQ, 512).transpose(2, 0, 1, 3)  # [NQ, KC, 128, 512]
    xi = xi.reshape(NQ, 4, 9, 128, 512).transpose(0, 1, 3, 2, 4)
    xi = np.ascontiguousarray(xi.reshape(NQ, 4, 128, QUART))
    in_maps = []
    for c in range(N_CORES):
        qs = slice(c * QH * 128, (c + 1) * QH * 128)
        kvs = slice(c * KVH * 128, (c + 1) * KVH * 128)
        wqt = wq[qs].T.astype(BF16_NP)   # [D, 512]
        wkt = wk[kvs].T.astype(BF16_NP)  # [D, 256]
        wvt = wv[kvs].T.astype(BF16_NP)
        wot = wo[:, qs].T.astype(BF16_NP)  # [512, D]
        wqi = wqt.reshape(KC, 128, 512).transpose(1, 0, 2)
        wqi = np.ascontiguousarray(
            wqi.reshape(128, 4, 9, 512).transpose(1, 0, 2, 3).reshape(4, 128, QUART))
        wki = wkt.reshape(KC, 128, 256).transpose(1, 0, 2)
        wki = np.ascontiguousarray(
            wki.reshape(128, 2, 18, 256).transpose(1, 0, 2, 3).reshape(2, 128, 18 * 256))
        wvi = wvt.reshape(KC, 128, 256).transpose(1, 0, 2)
        wvi = np.ascontiguousarray(
            wvi.reshape(128, 2, 18, 256).transpose(1, 0, 2, 3).reshape(2, 128, 18 * 256))
        wo4 = np.ascontiguousarray(wot.reshape(QH, 128, D))
        in_maps.append({
            "xq": xi,
            "wqq": wqi,
            "wkh": wki,
            "wvh": wvi,
            "wo4": wo4,
            "cost": cosT,
            "sint": sinT,
            "masks": masks,
        })
    return in_maps


def run(inputs, trace=False, trace_kwargs=None):
    from concourse.bass_utils import run_bass_kernel_spmd

    nc = build()
    x = np.asarray(inputs["x"], dtype=np.float32)
    in_maps = make_in_maps(
        x,
        np.asarray(inputs["wq"], dtype=np.float32),
        np.asarray(inputs["wk"], dtype=np.float32),
        np.asarray(inputs["wv"], dtype=np.float32),
        np.asarray(inputs["wo"], dtype=np.float32),
    )
    res = run_bass_kernel_spmd(
        nc, in_maps, core_ids=list(range(N_CORES)),
        trace=trace, **(trace_kwargs or {}))
    out = np.zeros((L, D), dtype=np.float32)
    for c in range(N_CORES):
        out += res.results[c]["out"].astype(np.float32)
    return out.reshape(x.shape), res


def kernel(**inputs) -> np.ndarray:
    out, _ = run(inputs, trace=False)
    return out


# revision 34
# speedup vs baseline: 1.2055x; 1.0097x over previous
"""Trainium2 Bass kernel for GQA attention (32 q heads / 16 kv heads, head_dim
128, L=2048, D=4608) with RoPE, tanh softcap 50, causal mask, o_proj.

Strategy: tensor-parallel over heads across 8 NeuronCores. Core c computes
q-heads 4c..4c+3 and kv-heads 2c..2c+1 end-to-end (QKV projections, RoPE,
softcapped causal attention, and the partial o_proj against its 512 columns of
wo); the host sums the 8 partial [L, D] outputs.

v13 design notes (v3 baseline 536-556us, v7 ~527us, v10 ~506us, v13
measures 502.6us; the changes target the measured losses: ~53us of
HAM-throttled time from phase-2 PE starvation, 12us startup, 9us of
early-phase-1 DMA waits, and scalar-engine congestion):
  - phase-1 chunk 0 is k-outer: the 36 contraction chunks stream Q(4)+K(2)
    matmuls into 6 parallel psum banks so x/wq DMA is consumed strictly in
    k (load) order; startup issues 512/1024-col wq/x/wk slivers first, so
    the first matmul starts as soon as the runtime DMA path opens (~10.5us,
    all framework preamble) and never outruns the loads. Chunks 1-3 are
    h-outer (x fully prefetched by then) so the six rope-drain chains
    stagger - bunching them at the phase boundary blocked the phase-2 PSUM
    pool handover behind ~14us of gpsimd work. Each chunk's V sweep covers
    its rope drains; chunk 3's V is deferred into phase 2 as PE filler.
    (Deferring chunk-3 K as well was tried and REGRESSED: its rope chain
    lands in the ACT+Pool queues mid-wave-1, delaying exp and the causal
    masks that PV waits on.)
  - per-pair exp (was per-quad): tanh f32-psum -> fp16 tt, exp tt -> pt
    immediately, halving the score->prob latency the PV units wait on.
  - phase-2 filler eligibility is pair-based: PV/transpose units of group g
    become eligible 2 score-pairs after g's last pair (was: 2 whole groups),
    so qA drains sooner and o_proj units unblock a group earlier - this
    removes the queue-starvation hole at nq boundaries that re-throttled the
    HAM clock gate (observed 10-17us half-clock windows). +3 pairs was tried
    and measured slightly worse. The pump budget per pair is wave-scoped
    adaptive: remaining queued filler cost divided by the pairs left in the
    CURRENT wave (clamped to [900, 3200]) - a fixed 2200ns/pair drained each
    wave's o_proj supply before its last groups; dividing by pairs left in
    the whole phase under-pumped mid-wave (both measured worse).
  - phase-2 waves run in nq order [1, 2, 0, 3] so each wave's tanh/exp
    latency is covered by exp-independent filler: wave 1 by the deferred-V
    units, wave 2 by fresh o_proj(1), thin wave 0 by o_proj(2) whose
    surplus carries into wave 3 alongside o_proj(0); the tail is o_proj(3),
    which is dense PE work. ([1,2,3,0] was tried and regressed badly: wave
    3 at group 8 reuses wave 1's double-buffered attnT tiles while
    o_proj(1) still holds reads on them.)
  - the pump_guard flushes deferred-V only at group >= 8 (not immediately -
    an earlier -10 tag made the guard dump all 15.6us of V filler before
    the first score pair, starving wave 1), and o_proj at group <= i-5 (its
    true attnT double-buffer hazard bound; i-3 front-loaded each wave's
    filler and starved the next wave's tail). PV(s) for s=0,1 only needs
    score tiles through the second-to-last pair, so it becomes eligible a
    pair earlier than s=2,3. Together these spread the filler into the
    exp-latency windows where the PE would otherwise idle and re-throttle
    (v7 -> v10 was -22us).
  - PV normalize reads the pv psum directly (DVE recip + scalar-mul);
    the scalar-engine staging copy is gone (19us off the ACT queue that
    tanh/exp share, and ~0.5us off each PV unit's latency).
  - PSUM->SBUF drains on DVE (GPSIMD/Pool cannot access PSUM); the SBUF-only
    causal-mask multiplies run on the otherwise-idle Pool engine.
  - o_proj psum pool is double-buffered; output staged and stored as bf16
    (host sums the 8 partials in f32); the final output row stores per-512
    so the closing transfer is small.
"""

from collections import deque

import numpy as np
import ml_dtypes

import concourse.bass as bass
import concourse.mybir as mybir
import concourse.tile as tile
from concourse.masks import make_identity
from concourse import bacc

F32 = mybir.dt.float32
FP16 = mybir.dt.float16
BF16 = mybir.dt.bfloat16
BF16_NP = ml_dtypes.bfloat16
AF = mybir.ActivationFunctionType

N_HEADS = 32
N_KV = 16
HEAD_DIM = 128
ROPE_THETA = 10000.0
SOFTCAP = 50.0
SCALE = 1.0 / 12.0  # 1/sqrt(144)
L = 2048
D = 4608
N_CORES = 8
QH = N_HEADS // N_CORES        # 4 local q heads
KVH = N_KV // N_CORES          # 2 local kv heads
KC = D // 128                  # 36 contraction chunks
NQ = L // 512                  # 4 l-chunks of 512
LT = L // 128                  # 16 l-tiles of 128
QUART = 9 * 512                # x/wq quarter width (9 k-chunks)


def _emit(nc):
    # DRAM tensors in SBUF-image layout (see make_in_maps)
    xq_d = nc.dram_tensor("xq", [NQ, 4, 128, QUART], BF16, kind="ExternalInput")
    wqq_d = nc.dram_tensor("wqq", [4, 128, QUART], BF16, kind="ExternalInput")
    wkh_d = nc.dram_tensor("wkh", [2, 128, 18 * 256], BF16, kind="ExternalInput")
    wvh_d = nc.dram_tensor("wvh", [2, 128, 18 * 256], BF16, kind="ExternalInput")
    wo4_d = nc.dram_tensor("wo4", [QH, 128, D], BF16, kind="ExternalInput")
    cost_d = nc.dram_tensor("cost", [128, L], BF16, kind="ExternalInput")
    sint_d = nc.dram_tensor("sint", [128, L], BF16, kind="ExternalInput")
    masks_d = nc.dram_tensor("masks", [128, 128], BF16, kind="ExternalInput")
    out_d = nc.dram_tensor("out", [L, D], BF16, kind="ExternalOutput")

    with tile.TileContext(nc) as tc:
        with (
            tc.tile_pool(name="const", bufs=1) as const,
            tc.tile_pool(name="persist", bufs=1) as persist,
            tc.tile_pool(name="xpb", bufs=1) as xpb,    # x chunks 1,3; chunk 3 outlives phase 1
            tc.tile_pool(name="wvp", bufs=1) as wvp,    # wv weights; outlive phase 1
        ):
            QT = [persist.tile([128, L], BF16, tag=f"qt{h}", name=f"qt{h}") for h in range(QH)]
            KT = [persist.tile([128, L], BF16, tag=f"kt{g}", name=f"kt{g}") for g in range(KVH)]
            # V extended with a ones column per k-tile: [128, 16*129]
            VE = [persist.tile([128, LT * 129], BF16, tag=f"ve{g}", name=f"ve{g}") for g in range(KVH)]

            ident = const.tile([128, 128], BF16)
            cost = const.tile([128, L], BF16)
            sint = const.tile([128, L], BF16)
            maskt = const.tile([128, 128], BF16)
            wvs = wvp.tile([128, KC * 256], BF16, name="wvs")

            deferred = _phase1(nc, tc, ident, cost, sint, maskt, wvs, QT, KT, VE,
                               xq_d, wqq_d, wkh_d, wvh_d, cost_d, sint_d,
                               masks_d, xpb)
            _phase2(nc, tc, ident, maskt, QT, KT, VE, wvs, wo4_d, out_d, deferred)
    return nc


def _phase1(nc, tc, ident, cost, sint, maskt, wvs, QT, KT, VE,
            xq_d, wqq_d, wkh_d, wvh_d, cost_d, sint_d, masks_d, xpb):
    with (
        tc.tile_pool(name="xpa", bufs=1) as xpa,
        tc.tile_pool(name="wqk", bufs=1) as wqk,
        tc.tile_pool(name="rtmp", bufs=3) as rtmp,
        tc.tile_pool(name="qk_psum", bufs=1, space="PSUM") as qk_psum,
        tc.tile_pool(name="vp_psum", bufs=2, space="PSUM") as vp_psum,
    ):
        wqs = wqk.tile([128, KC * 512], BF16, name="wqs")
        wks = wqk.tile([128, KC * 256], BF16, name="wks")

        # startup: k=0 slivers first so the first matmuls start ~3us in;
        # the rest lands in k-ascending order, always ahead of the k-outer
        # consumption (one 512-col x slice feeds 6 N=512 matmuls = 1.3us,
        # vs ~0.4us to transfer).
        x_first = xpa.tile([128, KC * 512], BF16, tag="xa", name="x0")
        nc.sync.dma_start(wqs[:, 0:512], wqq_d[0][:, 0:512])
        nc.sync.dma_start(x_first[:, 0:512], xq_d[0, 0][:, 0:512])
        nc.sync.dma_start(wks[:, 0:256], wkh_d[0][:, 0:256])
        nc.sync.dma_start(wqs[:, 512:1536], wqq_d[0][:, 512:1536])
        nc.sync.dma_start(x_first[:, 512:1536], xq_d[0, 0][:, 512:1536])
        nc.sync.dma_start(wks[:, 256:768], wkh_d[0][:, 256:768])
        nc.sync.dma_start(wqs[:, 1536:2560], wqq_d[0][:, 1536:2560])
        nc.sync.dma_start(x_first[:, 1536:2560], xq_d[0, 0][:, 1536:2560])
        nc.sync.dma_start(wqs[:, 2560:QUART], wqq_d[0][:, 2560:QUART])
        nc.sync.dma_start(x_first[:, 2560:QUART], xq_d[0, 0][:, 2560:QUART])
        nc.sync.dma_start(wks[:, 768:18 * 256], wkh_d[0][:, 768:18 * 256])
        nc.sync.dma_start(wqs[:, QUART:QUART + 2048], wqq_d[1][:, 0:2048])
        nc.sync.dma_start(x_first[:, QUART:QUART + 2048], xq_d[0, 1][:, 0:2048])
        nc.sync.dma_start(wqs[:, QUART + 2048:2 * QUART], wqq_d[1][:, 2048:QUART])
        nc.sync.dma_start(x_first[:, QUART + 2048:2 * QUART], xq_d[0, 1][:, 2048:QUART])
        for j in range(2, 4):
            nc.sync.dma_start(wqs[:, j * QUART:(j + 1) * QUART], wqq_d[j])
            nc.sync.dma_start(x_first[:, j * QUART:(j + 1) * QUART], xq_d[0, j])
        nc.sync.dma_start(wks[:, 18 * 256:36 * 256], wkh_d[1])
        nc.sync.dma_start(cost[:], cost_d[:])
        nc.sync.dma_start(sint[:], sint_d[:])
        for j in range(2):
            nc.sync.dma_start(wvs[:, j * 18 * 256:(j + 1) * 18 * 256], wvh_d[j])
        nc.sync.dma_start(maskt[:], masks_d[:])
        make_identity(nc, ident[:])

        def load_x(nq, dst):
            for j in range(4):
                nc.sync.dma_start(dst[:, j * QUART:(j + 1) * QUART], xq_d[nq, j])

        def drain_rope(ps, dst, nq, staged=False):
            """psum [128,512] f32 -> rope -> dst bf16 [128,512] slice.
            staged=True reads the psum once through a DVE copy so the bank
            (and at chunk 3, the whole phase-1 PSUM pool stack) releases
            after ~0.55us instead of after the 3-op read chain."""
            cols = slice(nq * 512, (nq + 1) * 512)
            if staged:
                stage = rtmp.tile([128, 512], F32, tag="stage")
                nc.vector.tensor_copy(stage[:], ps[:])
                ps = stage
            raw = rtmp.tile([128, 512], F32, tag="raw")
            nc.vector.tensor_mul(raw[:], ps[:], cost[:, cols])
            swap = rtmp.tile([128, 512], F32, tag="swap")
            nc.scalar.activation(swap[0:64, :], ps[64:128, :], AF.Copy)
            nc.scalar.activation(swap[64:128, :], ps[0:64, :], AF.Copy)
            nc.gpsimd.tensor_mul(swap[:], swap[:], sint[:, cols])
            nc.gpsimd.tensor_add(dst[:, cols], raw[:], swap[:])

        def alloc_x(nq):
            pool = xpb if nq % 2 == 1 else xpa
            tag = "xb" if nq % 2 == 1 else "xa"
            return pool.tile([128, KC * 512], BF16, tag=tag, name=f"x{nq}")

        xc = x_first
        for nq in range(NQ):
            if nq + 1 < NQ:
                xnext = alloc_x(nq + 1)
                load_x(nq + 1, xnext)
            if nq == 0:
                # k-outer: consumes x/wq strictly in k (DMA) order so the
                # startup never outruns the loads; drains bunch at the end
                # but overlap chunk 1's compute.
                psq = [qk_psum.tile([128, 512], F32, tag=f"q{h}", name=f"psq{h}")
                       for h in range(QH)]
                psk = [qk_psum.tile([128, 512], F32, tag=f"k{g}", name=f"psk{g}")
                       for g in range(KVH)]
                for k in range(KC):
                    for h in range(QH):
                        nc.tensor.matmul(
                            psq[h][:], wqs[:, k * 512 + h * 128:k * 512 + (h + 1) * 128],
                            xc[:, k * 512:(k + 1) * 512],
                            start=(k == 0), stop=(k == KC - 1))
                    for g in range(KVH):
                        nc.tensor.matmul(
                            psk[g][:], wks[:, k * 256 + g * 128:k * 256 + (g + 1) * 128],
                            xc[:, k * 512:(k + 1) * 512],
                            start=(k == 0), stop=(k == KC - 1))
                for h in range(QH):
                    drain_rope(psq[h], QT[h], nq)
                for g in range(KVH):
                    drain_rope(psk[g], KT[g], nq)
            else:
                # h-outer: x is fully prefetched by now; per-head psums drain
                # staggered so the engine queues (and the phase-2 handover for
                # chunk 3) never sit behind six bunched rope chains.
                for h in range(QH):
                    ps = qk_psum.tile([128, 512], F32, tag=f"q{h}", name=f"psq{h}")
                    for k in range(KC):
                        nc.tensor.matmul(
                            ps[:], wqs[:, k * 512 + h * 128:k * 512 + (h + 1) * 128],
                            xc[:, k * 512:(k + 1) * 512],
                            start=(k == 0), stop=(k == KC - 1))
                    drain_rope(ps, QT[h], nq, staged=(nq == NQ - 1))
                for g in range(KVH):
                    ps = qk_psum.tile([128, 512], F32, tag=f"k{g}", name=f"psk{g}")
                    for k in range(KC):
                        nc.tensor.matmul(
                            ps[:], wks[:, k * 256 + g * 128:k * 256 + (g + 1) * 128],
                            xc[:, k * 512:(k + 1) * 512],
                            start=(k == 0), stop=(k == KC - 1))
                    drain_rope(ps, KT[g], nq)
            if nq == NQ - 1:
                return [(nq, sub, xc) for sub in range(4)]  # V deferred to phase 2
            # V sweep of this chunk; its PE work covers the rope drains above
            for sub in range(4):
                mk = nq * 4 + sub
                ps = vp_psum.tile([128, KVH * 128], F32, tag="vps")
                for k in range(KC):
                    nc.tensor.matmul(
                        ps[:], xc[:, k * 512 + sub * 128:k * 512 + (sub + 1) * 128],
                        wvs[:, k * 256:(k + 1) * 256],
                        start=(k == 0), stop=(k == KC - 1))
                for g in range(KVH):
                    nc.vector.tensor_copy(
                        VE[g][:, mk * 129:mk * 129 + 128],
                        ps[:, g * 128:(g + 1) * 128])
                    nc.gpsimd.memset(VE[g][:, mk * 129 + 128:mk * 129 + 129], 1.0)
            xc = xnext


def _phase2(nc, tc, ident, maskt, QT, KT, VE, wvs, wo4_d, out_d, deferred_v):
    with (
        tc.tile_pool(name="wo", bufs=1) as wop,
        tc.tile_pool(name="pt", bufs=3) as ptp,
        tc.tile_pool(name="tt", bufs=2) as ttp,
        tc.tile_pool(name="attnt", bufs=2) as attp,
        tc.tile_pool(name="small", bufs=2) as small,
        tc.tile_pool(name="ostage", bufs=2) as ostage,
        tc.tile_pool(name="sc_psum", bufs=2, space="PSUM") as sc_psum,
        tc.tile_pool(name="pv_psum", bufs=1, space="PSUM") as pv_psum,
        tc.tile_pool(name="op_psum", bufs=2, space="PSUM") as op_psum,
        tc.tile_pool(name="atr_psum", bufs=1, space="PSUM") as atr_psum,
    ):
        WO = wop.tile([128, QH * D], BF16, name="wos")
        for h in range(QH):
            nc.sync.dma_start(WO[:, h * D:(h + 1) * D], wo4_d[h])

        # Wave order [1, 2, 0, 3]: each wave's softcap/exp latency is covered
        # by exp-independent PE filler - wave 1 by the deferred-V units (plus
        # its own larger score matmuls), wave 2 by fresh o_proj(1), thin wave
        # 0 by o_proj(2) whose surplus carries into wave 3 alongside
        # o_proj(0); the tail is o_proj(3), which is dense PE work. nq-major
        # order instead starved waves 2-3 (filler exhausted early) and
        # re-throttled the HAM clock gate.
        groups = [(nq, h) for nq in (1, 2, 0, 3) for h in range(QH)]
        state = {}
        att_of = {nq: [None] * QH for nq in range(NQ)}

        # Two filler queues:
        #   qA: exp-dependent units (PV / transpose), entries
        #       (elig_pair, group, pe_cost_ns, emit_fn) - eligible once the
        #       global score-pair counter passes 2 pairs beyond their group.
        #   qB: exp-independent units (o_proj, deferred V), entries
        #       (group, pe_cost_ns, emit_fn) - eligible once qA holds nothing
        #       at or before their group (preserves the T(nq,3) -> o_proj(nq)
        #       emission-order dependency).
        qA = deque()
        qB = deque()
        held_back = []            # final-flush filler (see end of function)
        pairs_done = [0]
        prefer_b = [False]
        queue_cost = [0]          # summed pe_cost_ns of everything queued

        def emit_next(i):
            """Emit one eligible filler unit; return its PE cost or None.
            The two queues alternate so a PV unit's psum-recycle (DVE
            recip+mul) never sits behind several o_proj drain casts in the
            DVE queue. qB entries with group None (deferred V) have no
            ordering dependency on qA at all."""
            a_ok = qA and qA[0][0] <= pairs_done[0]
            b_ok = qB and (qB[0][0] is None or not qA or qA[0][1] > qB[0][0])
            use_b = b_ok and (prefer_b[0] or not a_ok)
            if use_b:
                g, cost, fn = qB.popleft()
                prefer_b[0] = False
            elif a_ok:
                e, g, cost, fn = qA.popleft()
                prefer_b[0] = True
            else:
                return None
            fn()
            queue_cost[0] -= cost
            return cost

        def pump(i, pairs_left_in_wave):
            # spread the queued filler across the rest of this wave's score
            # pairs (+2 reserves a couple of units for the final flush): a
            # fixed 2200ns/pair drained each wave's o_proj supply before its
            # last groups, leaving them exp-gated in 1-3us clustered gaps
            # that re-throttle the HAM clock gate; spreading converts those
            # into harmless sub-window micro-gaps.
            ns = min(3200, max(900, queue_cost[0] // (pairs_left_in_wave + 2)))
            while ns > 0:
                c = emit_next(i)
                if c is None:
                    return
                ns -= c

        def pump_guard(i):
            # bound pipeline lag: PV/tr of group <= i-3 must be emitted
            # before group i reuses their pt buffer (pt pool is
            # triple-buffered). o_proj of wave W only conflicts with wave
            # W+2's attnT writes (attnt pool is double-buffered), so its
            # bound is i-5 - flushing it at i-3 front-loaded each wave's
            # filler and starved the next wave's tail. Deferred V units
            # (group None) are only forced once wave 3 approaches needing
            # VE; otherwise they interleave into wave 1's exp-latency
            # windows via pump().
            while qA and qA[0][1] <= i - 3:
                qA.popleft()[3]()
            while qB and ((qB[0][0] is None and i >= 8)
                          or (qB[0][0] is not None and qB[0][0] <= i - 5)):
                qB.popleft()[2]()

        def make_v_unit(vnq, sub, xc):
            def emit():
                mk = vnq * 4 + sub
                ps = op_psum.tile([128, 512], F32, tag="op")
                for k in range(KC):
                    nc.tensor.matmul(
                        ps[:, 0:KVH * 128],
                        xc[:, k * 512 + sub * 128:k * 512 + (sub + 1) * 128],
                        wvs[:, k * 256:(k + 1) * 256],
                        start=(k == 0), stop=(k == KC - 1))
                for g in range(KVH):
                    nc.vector.tensor_copy(
                        VE[g][:, mk * 129:mk * 129 + 128],
                        ps[:, g * 128:(g + 1) * 128])
                    nc.gpsimd.memset(VE[g][:, mk * 129 + 128:mk * 129 + 129], 1.0)
            return (None, 3900, emit)

        for u in deferred_v:
            unit = make_v_unit(*u)
            queue_cost[0] += unit[1]
            qB.append(unit)

        def emit_scores_pair(nq, h, pk, pt):
            """two score tiles into one [128,1024] f32 psum pair -> a single
            trimmed tanh into fp16 staging -> exp immediately (per-pair).
            Any stale psum between the two trimmed halves lands in pt columns
            that PV provably never reads."""
            g = h // 2
            ps = sc_psum.tile([128, 1024], F32, tag="sc")
            c0s = []
            for half in range(2):
                mk = 2 * pk + half
                o = mk - 4 * nq
                c0 = max(0, o) * 128
                c0s.append(c0)
                nc.tensor.matmul(
                    ps[:, half * 512 + c0:half * 512 + 512],
                    KT[g][:, mk * 128:(mk + 1) * 128],
                    QT[h][:, nq * 512 + c0:(nq + 1) * 512])
            tt = ttp.tile([128, 1024], FP16, tag="tt")
            nc.scalar.activation(
                tt[:, c0s[0]:1024], ps[:, c0s[0]:1024],
                AF.Tanh, scale=SCALE / SOFTCAP)
            nc.scalar.activation(
                pt[:, pk * 1024 + c0s[0]:(pk + 1) * 1024], tt[:, c0s[0]:1024],
                AF.Exp, scale=SOFTCAP)

        def emit_masks(nq, h, pt):
            # only the 128x128 block on the causal diagonal of each diagonal
            # tile is partially masked; everything left of it is never read
            for o in range(4):
                mk = 4 * nq + o
                base = mk * 512 + o * 128
                nc.gpsimd.tensor_mul(
                    pt[:, base:base + 128], pt[:, base:base + 128], maskt[:])

        def make_pv_unit(nq, h, s):
            def emit():
                st = state[(nq, h)]
                g = h // 2
                nks = 4 * nq + s + 1
                pv = pv_psum.tile([128, 129], F32, tag="pv")
                for mk in range(nks):
                    nc.tensor.matmul(
                        pv[:],
                        st["pt"][:, mk * 512 + s * 128:mk * 512 + (s + 1) * 128],
                        VE[g][:, mk * 129:(mk + 1) * 129],
                        start=(mk == 0), stop=(mk == nks - 1))
                recip = small.tile([128, 1], F32, tag="recip")
                nc.vector.reciprocal(recip[:], pv[:, 128:129])
                aq = small.tile([128, 128], BF16, tag=f"attnq{s}")
                nc.vector.tensor_scalar_mul(aq[:], pv[:, 0:128], recip[:])
                st["attnq"][s] = aq
            return 200 + (4 * nq + s + 1) * 110, emit

        def make_tr_unit(nq, h, s):
            def emit():
                st = state[(nq, h)]
                tp = atr_psum.tile([128, 128], BF16, tag="atr")
                nc.tensor.transpose(tp[:], st["attnq"][s][:], ident[:])
                nc.vector.tensor_copy(st["attnT"][:, s * 128:(s + 1) * 128], tp[:])
            return 180, emit

        def make_oproj_unit(idx, nq, s, j, ob, split_store=False):
            """single 512-wide o_proj chunk; DMA fires on each 3-chunk group
            boundary so at most one DVE drain sits between pumped PV units.
            The very last row stores per-512 so the final transfer is small."""
            def emit():
                po = op_psum.tile([128, 512], F32, tag="op")
                for h in range(QH):
                    nc.tensor.matmul(
                        po[:], att_of[nq][h][:, s * 128:(s + 1) * 128],
                        WO[:, h * D + j * 512:h * D + (j + 1) * 512],
                        start=(h == 0), stop=(h == QH - 1))
                nc.vector.tensor_copy(ob[:, (j % 3) * 512:(j % 3 + 1) * 512], po[:])
                row = nq * 512 + s * 128
                if split_store:
                    nc.sync.dma_start(
                        out_d[row:row + 128, j * 512:(j + 1) * 512],
                        ob[:, (j % 3) * 512:(j % 3 + 1) * 512])
                elif j % 3 == 2:
                    jg = j // 3
                    nc.sync.dma_start(
                        out_d[row:row + 128, jg * 1536:(jg + 1) * 1536], ob[:])
            return (idx, 1050, emit)

        for i, (nq, h) in enumerate(groups):
            npairs = 2 * nq + 2
            attnT = attp.tile([128, 512], BF16, tag=f"at{h}", name=f"at{h}")
            att_of[nq][h] = attnT
            pt = ptp.tile([128, LT * 512], BF16, tag="pt", name="pt")
            st = {"pt": pt, "attnq": [None] * 4, "attnT": attnT}
            state[(nq, h)] = st
            for pk in range(npairs):
                pump_guard(i)
                emit_scores_pair(nq, h, pk, pt)
                pairs_done[0] += 1
                pump(i, (4 - i % 4) * npairs - pk - 1)
            emit_masks(nq, h, pt)
            for s in range(4):
                # PV(s) only reads score tiles mk <= 4nq+s, so s=0,1 gate on
                # the second-to-last pair's exp and become eligible a pair
                # earlier than s=2,3.
                elig = pairs_done[0] + (1 if s < 2 else 2)
                cost, fn = make_pv_unit(nq, h, s)
                qA.append((elig, i, cost, fn))
                queue_cost[0] += cost
                cost, fn = make_tr_unit(nq, h, s)
                qA.append((elig, i, cost, fn))
                queue_cost[0] += cost
            if h == QH - 1:
                last_row = i == len(groups) - 1
                for s in range(4):
                    split = last_row and s == 3
                    ob = None
                    for j in range(D // 512):
                        if j % 3 == 0:
                            ob = ostage.tile([128, 1536], BF16, tag="ob")
                        unit = make_oproj_unit(i, nq, s, j, ob, split)
                        if i == len(groups) - 5 and s == 3 and j >= 6:
                            held_back.append(unit)  # final-flush exp cover
                        else:
                            queue_cost[0] += unit[1]
                            qB.append(unit)
        # cover the last group's exp latency with the held-back o_proj
        # units before its PV units hit the PE queue; they are from an
        # earlier wave, so their attnT reads are long satisfied, and they
        # emit before the last wave's o_proj so the ostage write-after-read
        # chain stays in order
        for g, cost, fn in held_back:
            fn()
        while qA:
            qA.popleft()[3]()
        while qB:
            qB.popleft()[2]()


_CACHED_NC = {}


def build():
    if "nc" not in _CACHED_NC:
        nc = bacc.Bacc("TRN2", target_bir_lowering=False, debug=False)
        _emit(nc)
        nc.compile()
        _CACHED_NC["nc"] = nc
    return _CACHED_NC["nc"]


def host_tables():
    inv_freq = 1.0 / (ROPE_THETA ** (np.arange(0, HEAD_DIM, 2, dtype=np.float32) / HEAD_DIM))
    ang = np.arange(L, dtype=np.float32)[:, None] * inv_freq[None, :]  # [L, 64]
    cos, sin = np.cos(ang), np.sin(ang)
    cosT = np.concatenate([cos.T, cos.T], axis=0).astype(BF16_NP)
    sinT = np.concatenate([-sin.T, sin.T], axis=0).astype(BF16_NP)
    return np.ascontiguousarray(cosT), np.ascontiguousarray(sinT)


def host_masks():
    k = np.arange(128)[:, None]
    q = np.arange(128)[None, :]
    return np.ascontiguousarray((q >= k).astype(BF16_NP))  # [128, 128] triangle


def make_in_maps(x, wq, wk, wv, wo):
    cosT, sinT = host_tables()
    masks = host_masks()
    xt = x.reshape(L, D).T.astype(BF16_NP)  # [D, L]
    # x image: [nq, j, p, k9*512+c] = xt[(9j+k9)*128+p, nq*512+c]
    xi = xt.reshape(KC, 128, NQ, 512).transpose(2, 0, 1, 3)  # [NQ, KC, 128, 512]
    xi = xi.reshape(NQ, 4, 9, 128, 512).transpose(0, 1, 3, 2, 4)
    xi = np.ascontiguousarray(xi.reshape(NQ, 4, 128, QUART))
    in_maps = []
    for c in range(N_CORES):
        qs = slice(c * QH * 128, (c + 1) * QH * 128)
        kvs = slice(c * KVH * 128, (c + 1) * KVH * 128)
        wqt = wq[qs].T.astype(BF16_NP)   # [D, 512]
        wkt = wk[kvs].T.astype(BF16_NP)  # [D, 256]
        wvt = wv[kvs].T.astype(BF16_NP)
        wot = wo[:, qs].T.astype(BF16_NP)  # [512, D]
        wqi = wqt.reshape(KC, 128, 512).transpose(1, 0, 2)
        wqi = np.ascontiguousarray(
            wqi.reshape(128, 4, 9, 512).transpose(1, 0, 2, 3).reshape(4, 128, QUART))
        wki = wkt.reshape(KC, 128, 256).transpose(1, 0, 2)
        wki = np.ascontiguousarray(
            wki.reshape(128, 2, 18, 256).transpose(1, 0, 2, 3).reshape(2, 128, 18 * 256))
        wvi = wvt.reshape(KC, 128, 256).transpose(1, 0, 2)
        wvi = np.ascontiguousarray(
            wvi.reshape(128, 2, 18, 256).transpose(1, 0, 2, 3).reshape(2, 128, 18 * 256))
        wo4 = np.ascontiguousarray(wot.reshape(QH, 128, D))
        in_maps.append({
            "xq": xi,
            "wqq": wqi,
            "wkh": wki,
            "wvh": wvi,
            "wo4": wo4,
            "cost": cosT,
            "sint": sinT,
            "masks": masks,
        })
    return in_maps


def run(inputs, trace=False, trace_kwargs=None):
    from concourse.bass_utils import run_bass_kernel_spmd

    nc = build()
    x = np.asarray(inputs["x"], dtype=np.float32)
    in_maps = make_in_maps(
        x,
        np.asarray(inputs["wq"], dtype=np.float32),
        np.asarray(inputs["wk"], dtype=np.float32),
        np.asarray(inputs["wv"], dtype=np.float32),
        np.asarray(inputs["wo"], dtype=np.float32),
    )
    res = run_bass_kernel_spmd(
        nc, in_maps, core_ids=list(range(N_CORES)),
        trace=trace, **(trace_kwargs or {}))
    out = np.zeros((L, D), dtype=np.float32)
    for c in range(N_CORES):
        out += res.results[c]["out"].astype(np.float32)
    return out.reshape(x.shape), res


def kernel(**inputs) -> np.ndarray:
    out, _ = run(inputs, trace=False)
    return out


# revision 36
# speedup vs baseline: 1.2068x; 1.0011x over previous
"""Trainium2 Bass kernel for GQA attention (32 q heads / 16 kv heads, head_dim
128, L=2048, D=4608) with RoPE, tanh softcap 50, causal mask, o_proj.

Strategy: tensor-parallel over heads across 8 NeuronCores. Core c computes
q-heads 4c..4c+3 and kv-heads 2c..2c+1 end-to-end (QKV projections, RoPE,
softcapped causal attention, and the partial o_proj against its 512 columns of
wo); the host sums the 8 partial [L, D] outputs.

v16 design notes (v3 baseline 536-556us, v7 ~527us, v10 ~506us, v15
~502us, v16 measures 497.5us; the changes target the measured losses:
~53us of HAM-throttled time from phase-2 PE starvation, 12us startup, 9us
of early-phase-1 DMA waits, and scalar-engine congestion):
  - phase-1 chunk 0 is k-outer: the 36 contraction chunks stream Q(4)+K(2)
    matmuls into 6 parallel psum banks so x/wq DMA is consumed strictly in
    k (load) order; startup issues 512/1024-col wq/x/wk slivers first, so
    the first matmul starts as soon as the runtime DMA path opens (~10.5us,
    all framework preamble) and never outruns the loads. Chunks 1-3 are
    h-outer (x fully prefetched by then) so the six rope-drain chains
    stagger - bunching them at the phase boundary blocked the phase-2 PSUM
    pool handover behind ~14us of gpsimd work. Each chunk's V sweep covers
    its rope drains; chunk 3's V is deferred into phase 2 as PE filler.
    (Deferring chunk-3 K as well was tried and REGRESSED: its rope chain
    lands in the ACT+Pool queues mid-wave-1, delaying exp and the causal
    masks that PV waits on.)
  - per-pair exp (was per-quad): tanh f32-psum -> fp16 tt, exp tt -> pt
    immediately, halving the score->prob latency the PV units wait on.
  - phase-2 filler eligibility is pair-based: PV/transpose units of group g
    become eligible 2 score-pairs after g's last pair (was: 2 whole groups),
    so qA drains sooner and o_proj units unblock a group earlier - this
    removes the queue-starvation hole at nq boundaries that re-throttled the
    HAM clock gate (observed 10-17us half-clock windows). +3 pairs was tried
    and measured slightly worse. The pump budget per pair is wave-scoped
    adaptive: remaining queued filler cost divided by the pairs left in the
    CURRENT wave (clamped to [900, 3200]) - a fixed 2200ns/pair drained each
    wave's o_proj supply before its last groups; dividing by pairs left in
    the whole phase under-pumped mid-wave (both measured worse).
  - phase-2 waves run in nq order [1, 2, 0, 3] so each wave's tanh/exp
    latency is covered by exp-independent filler: wave 1 by the deferred-V
    units, wave 2 by fresh o_proj(1), thin wave 0 by o_proj(2) whose
    surplus carries into wave 3 alongside o_proj(0); the tail is o_proj(3),
    which is dense PE work. ([1,2,3,0] was tried and regressed badly: wave
    3 at group 8 reuses wave 1's double-buffered attnT tiles while
    o_proj(1) still holds reads on them.)
  - the pump_guard flushes deferred-V only at group >= 8 (not immediately -
    an earlier -10 tag made the guard dump all 15.6us of V filler before
    the first score pair, starving wave 1), and o_proj at group <= i-5 (its
    true attnT double-buffer hazard bound; i-3 front-loaded each wave's
    filler and starved the next wave's tail). PV(s) for s=0,1 only needs
    score tiles through the second-to-last pair, so it becomes eligible a
    pair earlier than s=2,3. Together these spread the filler into the
    exp-latency windows where the PE would otherwise idle and re-throttle
    (v7 -> v10 was -22us).
  - PV normalize reads the pv psum directly (DVE recip + scalar-mul);
    the scalar-engine staging copy is gone (19us off the ACT queue that
    tanh/exp share, and ~0.5us off each PV unit's latency).
  - PSUM->SBUF drains on DVE (GPSIMD/Pool cannot access PSUM); the SBUF-only
    causal-mask multiplies run on the otherwise-idle Pool engine.
  - o_proj psum pool is double-buffered; output staged and stored as bf16
    (host sums the 8 partials in f32); the final output row stores per-512
    so the closing transfer is small.
  - v16: the last 3 o_proj units of the last safe wave are held back and
    emitted first in the final flush, covering the last group's ~3us exp
    latency (the pump otherwise drains the queue to zero); the j=1 x/wq
    quarter load is split so chunk-0's k-loop never outruns DMA; chunk-3's
    Q drains stage the psum through one DVE copy so the phase-2 PSUM pool
    handover starts ~1.5us earlier.
"""

from collections import deque

import numpy as np
import ml_dtypes

import concourse.bass as bass
import concourse.mybir as mybir
import concourse.tile as tile
from concourse.masks import make_identity
from concourse import bacc

F32 = mybir.dt.float32
FP16 = mybir.dt.float16
BF16 = mybir.dt.bfloat16
BF16_NP = ml_dtypes.bfloat16
AF = mybir.ActivationFunctionType

N_HEADS = 32
N_KV = 16
HEAD_DIM = 128
ROPE_THETA = 10000.0
SOFTCAP = 50.0
SCALE = 1.0 / 12.0  # 1/sqrt(144)
L = 2048
D = 4608
N_CORES = 8
QH = N_HEADS // N_CORES        # 4 local q heads
KVH = N_KV // N_CORES          # 2 local kv heads
KC = D // 128                  # 36 contraction chunks
NQ = L // 512                  # 4 l-chunks of 512
LT = L // 128                  # 16 l-tiles of 128
QUART = 9 * 512                # x/wq quarter width (9 k-chunks)


def _emit(nc):
    # DRAM tensors in SBUF-image layout (see make_in_maps)
    xq_d = nc.dram_tensor("xq", [NQ, 4, 128, QUART], BF16, kind="ExternalInput")
    wqq_d = nc.dram_tensor("wqq", [4, 128, QUART], BF16, kind="ExternalInput")
    wkh_d = nc.dram_tensor("wkh", [2, 128, 18 * 256], BF16, kind="ExternalInput")
    wvh_d = nc.dram_tensor("wvh", [2, 128, 18 * 256], BF16, kind="ExternalInput")
    wo4_d = nc.dram_tensor("wo4", [QH, 128, D], BF16, kind="ExternalInput")
    cost_d = nc.dram_tensor("cost", [128, L], BF16, kind="ExternalInput")
    sint_d = nc.dram_tensor("sint", [128, L], BF16, kind="ExternalInput")
    masks_d = nc.dram_tensor("masks", [128, 128], BF16, kind="ExternalInput")
    out_d = nc.dram_tensor("out", [L, D], BF16, kind="ExternalOutput")

    with tile.TileContext(nc) as tc:
        with (
            tc.tile_pool(name="const", bufs=1) as const,
            tc.tile_pool(name="persist", bufs=1) as persist,
            tc.tile_pool(name="xpb", bufs=1) as xpb,    # x chunks 1,3; chunk 3 outlives phase 1
            tc.tile_pool(name="wvp", bufs=1) as wvp,    # wv weights; outlive phase 1
        ):
            QT = [persist.tile([128, L], BF16, tag=f"qt{h}", name=f"qt{h}") for h in range(QH)]
            KT = [persist.tile([128, L], BF16, tag=f"kt{g}", name=f"kt{g}") for g in range(KVH)]
            # V extended with a ones column per k-tile: [128, 16*129]
            VE = [persist.tile([128, LT * 129], BF16, tag=f"ve{g}", name=f"ve{g}") for g in range(KVH)]

            ident = const.tile([128, 128], BF16)
            cost = const.tile([128, L], BF16)
            sint = const.tile([128, L], BF16)
            maskt = const.tile([128, 128], BF16)
            wvs = wvp.tile([128, KC * 256], BF16, name="wvs")

            deferred = _phase1(nc, tc, ident, cost, sint, maskt, wvs, QT, KT, VE,
                               xq_d, wqq_d, wkh_d, wvh_d, cost_d, sint_d,
                               masks_d, xpb)
            _phase2(nc, tc, ident, maskt, QT, KT, VE, wvs, wo4_d, out_d, deferred)
    return nc


def _phase1(nc, tc, ident, cost, sint, maskt, wvs, QT, KT, VE,
            xq_d, wqq_d, wkh_d, wvh_d, cost_d, sint_d, masks_d, xpb):
    with (
        tc.tile_pool(name="xpa", bufs=1) as xpa,
        tc.tile_pool(name="wqk", bufs=1) as wqk,
        tc.tile_pool(name="rtmp", bufs=3) as rtmp,
        tc.tile_pool(name="qk_psum", bufs=1, space="PSUM") as qk_psum,
        tc.tile_pool(name="vp_psum", bufs=2, space="PSUM") as vp_psum,
    ):
        wqs = wqk.tile([128, KC * 512], BF16, name="wqs")
        wks = wqk.tile([128, KC * 256], BF16, name="wks")

        # startup: k=0 slivers first so the first matmuls start ~3us in;
        # the rest lands in k-ascending order, always ahead of the k-outer
        # consumption (one 512-col x slice feeds 6 N=512 matmuls = 1.3us,
        # vs ~0.4us to transfer).
        x_first = xpa.tile([128, KC * 512], BF16, tag="xa", name="x0")
        nc.sync.dma_start(wqs[:, 0:512], wqq_d[0][:, 0:512])
        nc.sync.dma_start(x_first[:, 0:512], xq_d[0, 0][:, 0:512])
        nc.sync.dma_start(wks[:, 0:256], wkh_d[0][:, 0:256])
        nc.sync.dma_start(wqs[:, 512:1536], wqq_d[0][:, 512:1536])
        nc.sync.dma_start(x_first[:, 512:1536], xq_d[0, 0][:, 512:1536])
        nc.sync.dma_start(wks[:, 256:768], wkh_d[0][:, 256:768])
        nc.sync.dma_start(wqs[:, 1536:2560], wqq_d[0][:, 1536:2560])
        nc.sync.dma_start(x_first[:, 1536:2560], xq_d[0, 0][:, 1536:2560])
        nc.sync.dma_start(wqs[:, 2560:QUART], wqq_d[0][:, 2560:QUART])
        nc.sync.dma_start(x_first[:, 2560:QUART], xq_d[0, 0][:, 2560:QUART])
        nc.sync.dma_start(wks[:, 768:18 * 256], wkh_d[0][:, 768:18 * 256])
        nc.sync.dma_start(wqs[:, QUART:QUART + 2048], wqq_d[1][:, 0:2048])
        nc.sync.dma_start(x_first[:, QUART:QUART + 2048], xq_d[0, 1][:, 0:2048])
        nc.sync.dma_start(wqs[:, QUART + 2048:2 * QUART], wqq_d[1][:, 2048:QUART])
        nc.sync.dma_start(x_first[:, QUART + 2048:2 * QUART], xq_d[0, 1][:, 2048:QUART])
        for j in range(2, 4):
            nc.sync.dma_start(wqs[:, j * QUART:(j + 1) * QUART], wqq_d[j])
            nc.sync.dma_start(x_first[:, j * QUART:(j + 1) * QUART], xq_d[0, j])
        nc.sync.dma_start(wks[:, 18 * 256:36 * 256], wkh_d[1])
        nc.sync.dma_start(cost[:], cost_d[:])
        nc.sync.dma_start(sint[:], sint_d[:])
        for j in range(2):
            nc.sync.dma_start(wvs[:, j * 18 * 256:(j + 1) * 18 * 256], wvh_d[j])
        nc.sync.dma_start(maskt[:], masks_d[:])
        make_identity(nc, ident[:])

        def load_x(nq, dst):
            for j in range(4):
                nc.sync.dma_start(dst[:, j * QUART:(j + 1) * QUART], xq_d[nq, j])

        def drain_rope(ps, dst, nq, staged=False):
            """psum [128,512] f32 -> rope -> dst bf16 [128,512] slice.
            staged=True reads the psum once through a DVE copy so the bank
            (and at chunk 3, the whole phase-1 PSUM pool stack) releases
            after ~0.55us instead of after the 3-op read chain."""
            cols = slice(nq * 512, (nq + 1) * 512)
            if staged:
                stage = rtmp.tile([128, 512], F32, tag="stage")
                nc.vector.tensor_copy(stage[:], ps[:])
                ps = stage
            raw = rtmp.tile([128, 512], F32, tag="raw")
            nc.vector.tensor_mul(raw[:], ps[:], cost[:, cols])
            swap = rtmp.tile([128, 512], F32, tag="swap")
            nc.scalar.activation(swap[0:64, :], ps[64:128, :], AF.Copy)
            nc.scalar.activation(swap[64:128, :], ps[0:64, :], AF.Copy)
            nc.gpsimd.tensor_mul(swap[:], swap[:], sint[:, cols])
            nc.gpsimd.tensor_add(dst[:, cols], raw[:], swap[:])

        def alloc_x(nq):
            pool = xpb if nq % 2 == 1 else xpa
            tag = "xb" if nq % 2 == 1 else "xa"
            return pool.tile([128, KC * 512], BF16, tag=tag, name=f"x{nq}")

        xc = x_first
        for nq in range(NQ):
            if nq + 1 < NQ:
                xnext = alloc_x(nq + 1)
                load_x(nq + 1, xnext)
            if nq == 0:
                # k-outer: consumes x/wq strictly in k (DMA) order so the
                # startup never outruns the loads; drains bunch at the end
                # but overlap chunk 1's compute.
                psq = [qk_psum.tile([128, 512], F32, tag=f"q{h}", name=f"psq{h}")
                       for h in range(QH)]
                psk = [qk_psum.tile([128, 512], F32, tag=f"k{g}", name=f"psk{g}")
                       for g in range(KVH)]
                for k in range(KC):
                    for h in range(QH):
                        nc.tensor.matmul(
                            psq[h][:], wqs[:, k * 512 + h * 128:k * 512 + (h + 1) * 128],
                            xc[:, k * 512:(k + 1) * 512],
                            start=(k == 0), stop=(k == KC - 1))
                    for g in range(KVH):
                        nc.tensor.matmul(
                            psk[g][:], wks[:, k * 256 + g * 128:k * 256 + (g + 1) * 128],
                            xc[:, k * 512:(k + 1) * 512],
                            start=(k == 0), stop=(k == KC - 1))
                for h in range(QH):
                    drain_rope(psq[h], QT[h], nq)
                for g in range(KVH):
                    drain_rope(psk[g], KT[g], nq)
            else:
                # h-outer: x is fully prefetched by now; per-head psums drain
                # staggered so the engine queues (and the phase-2 handover for
                # chunk 3) never sit behind six bunched rope chains.
                for h in range(QH):
                    ps = qk_psum.tile([128, 512], F32, tag=f"q{h}", name=f"psq{h}")
                    for k in range(KC):
                        nc.tensor.matmul(
                            ps[:], wqs[:, k * 512 + h * 128:k * 512 + (h + 1) * 128],
                            xc[:, k * 512:(k + 1) * 512],
                            start=(k == 0), stop=(k == KC - 1))
                    drain_rope(ps, QT[h], nq, staged=(nq == NQ - 1))
                for g in range(KVH):
                    ps = qk_psum.tile([128, 512], F32, tag=f"k{g}", name=f"psk{g}")
                    for k in range(KC):
                        nc.tensor.matmul(
                            ps[:], wks[:, k * 256 + g * 128:k * 256 + (g + 1) * 128],
                            xc[:, k * 512:(k + 1) * 512],
                            start=(k == 0), stop=(k == KC - 1))
                    drain_rope(ps, KT[g], nq)
            if nq == NQ - 1:
                return [(nq, sub, xc) for sub in range(4)]  # V deferred to phase 2
            # V sweep of this chunk; its PE work covers the rope drains above
            for sub in range(4):
                mk = nq * 4 + sub
                ps = vp_psum.tile([128, KVH * 128], F32, tag="vps")
                for k in range(KC):
                    nc.tensor.matmul(
                        ps[:], xc[:, k * 512 + sub * 128:k * 512 + (sub + 1) * 128],
                        wvs[:, k * 256:(k + 1) * 256],
                        start=(k == 0), stop=(k == KC - 1))
                for g in range(KVH):
                    nc.vector.tensor_copy(
                        VE[g][:, mk * 129:mk * 129 + 128],
                        ps[:, g * 128:(g + 1) * 128])
                    nc.gpsimd.memset(VE[g][:, mk * 129 + 128:mk * 129 + 129], 1.0)
            xc = xnext


def _phase2(nc, tc, ident, maskt, QT, KT, VE, wvs, wo4_d, out_d, deferred_v):
    with (
        tc.tile_pool(name="wo", bufs=1) as wop,
        tc.tile_pool(name="pt", bufs=3) as ptp,
        tc.tile_pool(name="tt", bufs=2) as ttp,
        tc.tile_pool(name="attnt", bufs=2) as attp,
        tc.tile_pool(name="small", bufs=2) as small,
        tc.tile_pool(name="ostage", bufs=2) as ostage,
        tc.tile_pool(name="sc_psum", bufs=2, space="PSUM") as sc_psum,
        tc.tile_pool(name="pv_psum", bufs=1, space="PSUM") as pv_psum,
        tc.tile_pool(name="op_psum", bufs=2, space="PSUM") as op_psum,
        tc.tile_pool(name="atr_psum", bufs=1, space="PSUM") as atr_psum,
    ):
        WO = wop.tile([128, QH * D], BF16, name="wos")
        for h in range(QH):
            nc.sync.dma_start(WO[:, h * D:(h + 1) * D], wo4_d[h])

        # Wave order [1, 2, 0, 3]: each wave's softcap/exp latency is covered
        # by exp-independent PE filler - wave 1 by the deferred-V units (plus
        # its own larger score matmuls), wave 2 by fresh o_proj(1), thin wave
        # 0 by o_proj(2) whose surplus carries into wave 3 alongside
        # o_proj(0); the tail is o_proj(3), which is dense PE work. nq-major
        # order instead starved waves 2-3 (filler exhausted early) and
        # re-throttled the HAM clock gate.
        groups = [(nq, h) for nq in (1, 2, 0, 3) for h in range(QH)]
        state = {}
        att_of = {nq: [None] * QH for nq in range(NQ)}

        # Two filler queues:
        #   qA: exp-dependent units (PV / transpose), entries
        #       (elig_pair, group, pe_cost_ns, emit_fn) - eligible once the
        #       global score-pair counter passes 2 pairs beyond their group.
        #   qB: exp-independent units (o_proj, deferred V), entries
        #       (group, pe_cost_ns, emit_fn) - eligible once qA holds nothing
        #       at or before their group (preserves the T(nq,3) -> o_proj(nq)
        #       emission-order dependency).
        qA = deque()
        qB = deque()
        held_back = []            # final-flush filler (see end of function)
        pairs_done = [0]
        prefer_b = [False]
        queue_cost = [0]          # summed pe_cost_ns of everything queued

        def emit_next(i):
            """Emit one eligible filler unit; return its PE cost or None.
            The two queues alternate so a PV unit's psum-recycle (DVE
            recip+mul) never sits behind several o_proj drain casts in the
            DVE queue. qB entries with group None (deferred V) have no
            ordering dependency on qA at all."""
            a_ok = qA and qA[0][0] <= pairs_done[0]
            b_ok = qB and (qB[0][0] is None or not qA or qA[0][1] > qB[0][0])
            use_b = b_ok and (prefer_b[0] or not a_ok)
            if use_b:
                g, cost, fn = qB.popleft()
                prefer_b[0] = False
            elif a_ok:
                e, g, cost, fn = qA.popleft()
                prefer_b[0] = True
            else:
                return None
            fn()
            queue_cost[0] -= cost
            return cost

        def pump(i, pairs_left_in_wave):
            # spread the queued filler across the rest of this wave's score
            # pairs (+2 reserves a couple of units for the final flush): a
            # fixed 2200ns/pair drained each wave's o_proj supply before its
            # last groups, leaving them exp-gated in 1-3us clustered gaps
            # that re-throttle the HAM clock gate; spreading converts those
            # into harmless sub-window micro-gaps.
            ns = min(3200, max(900, queue_cost[0] // (pairs_left_in_wave + 2)))
            while ns > 0:
                c = emit_next(i)
                if c is None:
                    return
                ns -= c

        def pump_guard(i):
            # bound pipeline lag: PV/tr of group <= i-3 must be emitted
            # before group i reuses their pt buffer (pt pool is
            # triple-buffered). o_proj of wave W only conflicts with wave
            # W+2's attnT writes (attnt pool is double-buffered), so its
            # bound is i-5 - flushing it at i-3 front-loaded each wave's
            # filler and starved the next wave's tail. Deferred V units
            # (group None) are only forced once wave 3 approaches needing
            # VE; otherwise they interleave into wave 1's exp-latency
            # windows via pump().
            while qA and qA[0][1] <= i - 3:
                qA.popleft()[3]()
            while qB and ((qB[0][0] is None and i >= 8)
                          or (qB[0][0] is not None and qB[0][0] <= i - 5)):
                qB.popleft()[2]()

        def make_v_unit(vnq, sub, xc):
            def emit():
                mk = vnq * 4 + sub
                ps = op_psum.tile([128, 512], F32, tag="op")
                for k in range(KC):
                    nc.tensor.matmul(
                        ps[:, 0:KVH * 128],
                        xc[:, k * 512 + sub * 128:k * 512 + (sub + 1) * 128],
                        wvs[:, k * 256:(k + 1) * 256],
                        start=(k == 0), stop=(k == KC - 1))
                for g in range(KVH):
                    nc.vector.tensor_copy(
                        VE[g][:, mk * 129:mk * 129 + 128],
                        ps[:, g * 128:(g + 1) * 128])
                    nc.gpsimd.memset(VE[g][:, mk * 129 + 128:mk * 129 + 129], 1.0)
            return (None, 3900, emit)

        for u in deferred_v:
            unit = make_v_unit(*u)
            queue_cost[0] += unit[1]
            qB.append(unit)

        def emit_scores_pair(nq, h, pk, pt):
            """two score tiles into one [128,1024] f32 psum pair -> a single
            trimmed tanh into fp16 staging -> exp immediately (per-pair).
            Any stale psum between the two trimmed halves lands in pt columns
            that PV provably never reads."""
            g = h // 2
            ps = sc_psum.tile([128, 1024], F32, tag="sc")
            c0s = []
            for half in range(2):
                mk = 2 * pk + half
                o = mk - 4 * nq
                c0 = max(0, o) * 128
                c0s.append(c0)
                nc.tensor.matmul(
                    ps[:, half * 512 + c0:half * 512 + 512],
                    KT[g][:, mk * 128:(mk + 1) * 128],
                    QT[h][:, nq * 512 + c0:(nq + 1) * 512])
            tt = ttp.tile([128, 1024], FP16, tag="tt")
            nc.scalar.activation(
                tt[:, c0s[0]:1024], ps[:, c0s[0]:1024],
                AF.Tanh, scale=SCALE / SOFTCAP)
            nc.scalar.activation(
                pt[:, pk * 1024 + c0s[0]:(pk + 1) * 1024], tt[:, c0s[0]:1024],
                AF.Exp, scale=SOFTCAP)

        def emit_masks(nq, h, pt):
            # only the 128x128 block on the causal diagonal of each diagonal
            # tile is partially masked; everything left of it is never read
            for o in range(4):
                mk = 4 * nq + o
                base = mk * 512 + o * 128
                nc.gpsimd.tensor_mul(
                    pt[:, base:base + 128], pt[:, base:base + 128], maskt[:])

        def make_pv_unit(nq, h, s):
            def emit():
                st = state[(nq, h)]
                g = h // 2
                nks = 4 * nq + s + 1
                pv = pv_psum.tile([128, 129], F32, tag="pv")
                for mk in range(nks):
                    nc.tensor.matmul(
                        pv[:],
                        st["pt"][:, mk * 512 + s * 128:mk * 512 + (s + 1) * 128],
                        VE[g][:, mk * 129:(mk + 1) * 129],
                        start=(mk == 0), stop=(mk == nks - 1))
                recip = small.tile([128, 1], F32, tag="recip")
                nc.vector.reciprocal(recip[:], pv[:, 128:129])
                aq = small.tile([128, 128], BF16, tag=f"attnq{s}")
                nc.vector.tensor_scalar_mul(aq[:], pv[:, 0:128], recip[:])
                st["attnq"][s] = aq
            return 200 + (4 * nq + s + 1) * 110, emit

        def make_tr_unit(nq, h, s):
            def emit():
                st = state[(nq, h)]
                tp = atr_psum.tile([128, 128], BF16, tag="atr")
                nc.tensor.transpose(tp[:], st["attnq"][s][:], ident[:])
                nc.vector.tensor_copy(st["attnT"][:, s * 128:(s + 1) * 128], tp[:])
            return 180, emit

        def make_oproj_unit(idx, nq, s, j, ob, split_store=False):
            """single 512-wide o_proj chunk; DMA fires on each 3-chunk group
            boundary so at most one DVE drain sits between pumped PV units.
            The very last row stores per-512 so the final transfer is small.
            Units that only ever run in the tail (the last wave's, and the
            held-back final-flush cover) drain on the scalar engine - idle
            there, and it reads PSUM faster than DVE does."""
            tail_time = nq == 3 or (idx == len(groups) - 5 and s == 3 and j >= 3)
            def emit():
                po = op_psum.tile([128, 512], F32, tag="op")
                for h in range(QH):
                    nc.tensor.matmul(
                        po[:], att_of[nq][h][:, s * 128:(s + 1) * 128],
                        WO[:, h * D + j * 512:h * D + (j + 1) * 512],
                        start=(h == 0), stop=(h == QH - 1))
                dst = ob[:, (j % 3) * 512:(j % 3 + 1) * 512]
                if tail_time:
                    nc.scalar.activation(dst, po[:], AF.Copy)
                else:
                    nc.vector.tensor_copy(dst, po[:])
                row = nq * 512 + s * 128
                if split_store:
                    nc.sync.dma_start(
                        out_d[row:row + 128, j * 512:(j + 1) * 512],
                        ob[:, (j % 3) * 512:(j % 3 + 1) * 512])
                elif j % 3 == 2:
                    jg = j // 3
                    nc.sync.dma_start(
                        out_d[row:row + 128, jg * 1536:(jg + 1) * 1536], ob[:])
            return (idx, 1050, emit)

        for i, (nq, h) in enumerate(groups):
            npairs = 2 * nq + 2
            attnT = attp.tile([128, 512], BF16, tag=f"at{h}", name=f"at{h}")
            att_of[nq][h] = attnT
            pt = ptp.tile([128, LT * 512], BF16, tag="pt", name="pt")
            st = {"pt": pt, "attnq": [None] * 4, "attnT": attnT}
            state[(nq, h)] = st
            for pk in range(npairs):
                pump_guard(i)
                emit_scores_pair(nq, h, pk, pt)
                pairs_done[0] += 1
                pump(i, (4 - i % 4) * npairs - pk - 1)
            emit_masks(nq, h, pt)
            for s in range(4):
                # PV(s) only reads score tiles mk <= 4nq+s, so s=0,1 gate on
                # the second-to-last pair's exp and become eligible a pair
                # earlier than s=2,3. Wave-1 groups (i<4) wait one pair
                # longer: their filler (deferred V) bypasses the qA ordering
                # check, so late eligibility costs nothing, and emitting PV
                # before its exp cleared was measured as 0.7-1us PE waits.
                elig = pairs_done[0] + (1 if s < 2 else 2) + (1 if i < 4 else 0)
                cost, fn = make_pv_unit(nq, h, s)
                qA.append((elig, i, cost, fn))
                queue_cost[0] += cost
                cost, fn = make_tr_unit(nq, h, s)
                qA.append((elig, i, cost, fn))
                queue_cost[0] += cost
            if h == QH - 1:
                last_row = i == len(groups) - 1
                for s in range(4):
                    split = last_row and s == 3
                    ob = None
                    for j in range(D // 512):
                        if j % 3 == 0:
                            ob = ostage.tile([128, 1536], BF16, tag="ob")
                        unit = make_oproj_unit(i, nq, s, j, ob, split)
                        if i == len(groups) - 5 and s == 3 and j >= 3:
                            held_back.append(unit)  # final-flush exp cover
                        else:
                            queue_cost[0] += unit[1]
                            qB.append(unit)
        # cover the last group's exp latency with the held-back o_proj
        # units before its PV units hit the PE queue; they are from an
        # earlier wave, so their attnT reads are long satisfied, and they
        # emit before the last wave's o_proj so the ostage write-after-read
        # chain stays in order
        for g, cost, fn in held_back:
            fn()
        while qA:
            qA.popleft()[3]()
        while qB:
            qB.popleft()[2]()


_CACHED_NC = {}


def build():
    if "nc" not in _CACHED_NC:
        nc = bacc.Bacc("TRN2", target_bir_lowering=False, debug=False)
        _emit(nc)
        nc.compile()
        _CACHED_NC["nc"] = nc
    return _CACHED_NC["nc"]


def host_tables():
    inv_freq = 1.0 / (ROPE_THETA ** (np.arange(0, HEAD_DIM, 2, dtype=np.float32) / HEAD_DIM))
    ang = np.arange(L, dtype=np.float32)[:, None] * inv_freq[None, :]  # [L, 64]
    cos, sin = np.cos(ang), np.sin(ang)
    cosT = np.concatenate([cos.T, cos.T], axis=0).astype(BF16_NP)
    sinT = np.concatenate([-sin.T, sin.T], axis=0).astype(BF16_NP)
    return np.ascontiguousarray(cosT), np.ascontiguousarray(sinT)


def host_masks():
    k = np.arange(128)[:, None]
    q = np.arange(128)[None, :]
    return np.ascontiguousarray((q >= k).astype(BF16_NP))  # [128, 128] triangle


def make_in_maps(x, wq, wk, wv, wo):
    cosT, sinT = host_tables()
    masks = host_masks()
    xt = x.reshape(L, D).T.astype(BF16_NP)  # [D, L]
    # x image: [nq, j, p, k9*512+c] = xt[(9j+k9)*128+p, nq*512+c]
    xi = xt.reshape(KC, 128, NQ, 512).transpose(2, 0, 1, 3)  # [NQ, KC, 128, 512]
    xi = xi.reshape(NQ, 4, 9, 128, 512).transpose(0, 1, 3, 2, 4)
    xi = np.ascontiguousarray(xi.reshape(NQ, 4, 128, QUART))
    in_maps = []
    for c in range(N_CORES):
        qs = slice(c * QH * 128, (c + 1) * QH * 128)
        kvs = slice(c * KVH * 128, (c + 1) * KVH * 128)
        wqt = wq[qs].T.astype(BF16_NP)   # [D, 512]
        wkt = wk[kvs].T.astype(BF16_NP)  # [D, 256]
        wvt = wv[kvs].T.astype(BF16_NP)
        wot = wo[:, qs].T.astype(BF16_NP)  # [512, D]
        wqi = wqt.reshape(KC, 128, 512).transpose(1, 0, 2)
        wqi = np.ascontiguousarray(
            wqi.reshape(128, 4, 9, 512).transpose(1, 0, 2, 3).reshape(4, 128, QUART))
        wki = wkt.reshape(KC, 128, 256).transpose(1, 0, 2)
        wki = np.ascontiguousarray(
            wki.reshape(128, 2, 18, 256).transpose(1, 0, 2, 3).reshape(2, 128, 18 * 256))
        wvi = wvt.reshape(KC, 128, 256).transpose(1, 0, 2)
        wvi = np.ascontiguousarray(
            wvi.reshape(128, 2, 18, 256).transpose(1, 0, 2, 3).reshape(2, 128, 18 * 256))
        wo4 = np.ascontiguousarray(wot.reshape(QH, 128, D))
        in_maps.append({
            "xq": xi,
            "wqq": wqi,
            "wkh": wki,
            "wvh": wvi,
            "wo4": wo4,
            "cost": cosT,
            "sint": sinT,
            "masks": masks,
        })
    return in_maps


def run(inputs, trace=False, trace_kwargs=None):
    from concourse.bass_utils import run_bass_kernel_spmd

    nc = build()
    x = np.asarray(inputs["x"], dtype=np.float32)
    in_maps = make_in_maps(
        x,
        np.asarray(inputs["wq"], dtype=np.float32),
        np.asarray(inputs["wk"], dtype=np.float32),
        np.asarray(inputs["wv"], dtype=np.float32),
        np.asarray(inputs["wo"], dtype=np.float32),
    )
    res = run_bass_kernel_spmd(
        nc, in_maps, core_ids=list(range(N_CORES)),
        trace=trace, **(trace_kwargs or {}))
    out = np.zeros((L, D), dtype=np.float32)
    for c in range(N_CORES):
        out += res.results[c]["out"].astype(np.float32)
    return out.reshape(x.shape), res


def kernel(**inputs) -> np.ndarray:
    out, _ = run(inputs, trace=False)
    return out


# revision 38
# speedup vs baseline: 1.2156x; 1.0073x over previous
"""Trainium2 Bass kernel for GQA attention (32 q heads / 16 kv heads, head_dim
128, L=2048, D=4608) with RoPE, tanh softcap 50, causal mask, o_proj.

Strategy: tensor-parallel over heads across 8 NeuronCores. Core c computes
q-heads 4c..4c+3 and kv-heads 2c..2c+1 end-to-end (QKV projections, RoPE,
softcapped causal attention, and the partial o_proj against its 512 columns of
wo); the host sums the 8 partial [L, D] outputs.

v17 design notes (v3 baseline 536-556us, v7 ~527us, v10 ~506us, v15
~502us, v16 497.5us, v17 measures 497.0us; the changes target the measured losses:
~53us of HAM-throttled time from phase-2 PE starvation, 12us startup, 9us
of early-phase-1 DMA waits, and scalar-engine congestion):
  - phase-1 chunk 0 is k-outer: the 36 contraction chunks stream Q(4)+K(2)
    matmuls into 6 parallel psum banks so x/wq DMA is consumed strictly in
    k (load) order; startup issues 512/1024-col wq/x/wk slivers first, so
    the first matmul starts as soon as the runtime DMA path opens (~10.5us,
    all framework preamble) and never outruns the loads. Chunks 1-3 are
    h-outer (x fully prefetched by then) so the six rope-drain chains
    stagger - bunching them at the phase boundary blocked the phase-2 PSUM
    pool handover behind ~14us of gpsimd work. Each chunk's V sweep covers
    its rope drains; chunk 3's V is deferred into phase 2 as PE filler.
    (Deferring chunk-3 K as well was tried and REGRESSED: its rope chain
    lands in the ACT+Pool queues mid-wave-1, delaying exp and the causal
    masks that PV waits on.)
  - per-pair exp (was per-quad): tanh f32-psum -> fp16 tt, exp tt -> pt
    immediately, halving the score->prob latency the PV units wait on.
  - phase-2 filler eligibility is pair-based: PV/transpose units of group g
    become eligible 2 score-pairs after g's last pair (was: 2 whole groups),
    so qA drains sooner and o_proj units unblock a group earlier - this
    removes the queue-starvation hole at nq boundaries that re-throttled the
    HAM clock gate (observed 10-17us half-clock windows). +3 pairs was tried
    and measured slightly worse. The pump budget per pair is wave-scoped
    adaptive: remaining queued filler cost divided by the pairs left in the
    CURRENT wave (clamped to [900, 3200]) - a fixed 2200ns/pair drained each
    wave's o_proj supply before its last groups; dividing by pairs left in
    the whole phase under-pumped mid-wave (both measured worse).
  - phase-2 waves run in nq order [1, 2, 0, 3] so each wave's tanh/exp
    latency is covered by exp-independent filler: wave 1 by the deferred-V
    units, wave 2 by fresh o_proj(1), thin wave 0 by o_proj(2) whose
    surplus carries into wave 3 alongside o_proj(0); the tail is o_proj(3),
    which is dense PE work. ([1,2,3,0] was tried and regressed badly: wave
    3 at group 8 reuses wave 1's double-buffered attnT tiles while
    o_proj(1) still holds reads on them.)
  - the pump_guard flushes deferred-V only at group >= 8 (not immediately -
    an earlier -10 tag made the guard dump all 15.6us of V filler before
    the first score pair, starving wave 1), and o_proj at group <= i-5 (its
    true attnT double-buffer hazard bound; i-3 front-loaded each wave's
    filler and starved the next wave's tail). PV(s) for s=0,1 only needs
    score tiles through the second-to-last pair, so it becomes eligible a
    pair earlier than s=2,3. Together these spread the filler into the
    exp-latency windows where the PE would otherwise idle and re-throttle
    (v7 -> v10 was -22us).
  - PV normalize reads the pv psum directly (DVE recip + scalar-mul);
    the scalar-engine staging copy is gone (19us off the ACT queue that
    tanh/exp share, and ~0.5us off each PV unit's latency).
  - PSUM->SBUF drains on DVE (GPSIMD/Pool cannot access PSUM); the SBUF-only
    causal-mask multiplies run on the otherwise-idle Pool engine.
  - o_proj psum pool is double-buffered; output staged and stored as bf16
    (host sums the 8 partials in f32); the final output row stores per-512
    so the closing transfer is small.
  - v16/v17: the last 6 o_proj units of the last safe wave are held back
    and emitted first in the final flush, covering the last group's exp
    latency plus the wave-3 ACT queue backlog (the pump otherwise drains
    the queue to zero); the j=1 x/wq quarter load is split so chunk-0's
    k-loop never outruns DMA; chunk-3's Q drains stage the psum through one
    DVE copy so the phase-2 PSUM pool handover starts earlier; tail-time
    o_proj units (last wave's + held-back) drain on the scalar engine
    (idle in the tail, faster PSUM reads than DVE); wave-1 groups' PV
    eligibility is one pair later (their V filler bypasses the qA ordering
    check, so late eligibility is free there).
"""

from collections import deque

import numpy as np
import ml_dtypes

import concourse.bass as bass
import concourse.mybir as mybir
import concourse.tile as tile
from concourse.masks import make_identity
from concourse import bacc

F32 = mybir.dt.float32
FP16 = mybir.dt.float16
BF16 = mybir.dt.bfloat16
BF16_NP = ml_dtypes.bfloat16
AF = mybir.ActivationFunctionType

N_HEADS = 32
N_KV = 16
HEAD_DIM = 128
ROPE_THETA = 10000.0
SOFTCAP = 50.0
SCALE = 1.0 / 12.0  # 1/sqrt(144)
L = 2048
D = 4608
N_CORES = 8
QH = N_HEADS // N_CORES        # 4 local q heads
KVH = N_KV // N_CORES          # 2 local kv heads
KC = D // 128                  # 36 contraction chunks
NQ = L // 512                  # 4 l-chunks of 512
LT = L // 128                  # 16 l-tiles of 128
QUART = 9 * 512                # x/wq quarter width (9 k-chunks)


def _emit(nc):
    # DRAM tensors in SBUF-image layout (see make_in_maps)
    xq_d = nc.dram_tensor("xq", [NQ, 4, 128, QUART], BF16, kind="ExternalInput")
    wqq_d = nc.dram_tensor("wqq", [4, 128, QUART], BF16, kind="ExternalInput")
    wkh_d = nc.dram_tensor("wkh", [2, 128, 18 * 256], BF16, kind="ExternalInput")
    wvh_d = nc.dram_tensor("wvh", [2, 128, 18 * 256], BF16, kind="ExternalInput")
    wo4_d = nc.dram_tensor("wo4", [QH, 128, D], BF16, kind="ExternalInput")
    cost_d = nc.dram_tensor("cost", [128, L], BF16, kind="ExternalInput")
    sint_d = nc.dram_tensor("sint", [128, L], BF16, kind="ExternalInput")
    masks_d = nc.dram_tensor("masks", [128, 128], BF16, kind="ExternalInput")
    out_d = nc.dram_tensor("out", [L, D], BF16, kind="ExternalOutput")

    with tile.TileContext(nc) as tc:
        with (
            tc.tile_pool(name="const", bufs=1) as const,
            tc.tile_pool(name="persist", bufs=1) as persist,
            tc.tile_pool(name="xpb", bufs=1) as xpb,    # x chunks 1,3; chunk 3 outlives phase 1
            tc.tile_pool(name="wvp", bufs=1) as wvp,    # wv weights; outlive phase 1
        ):
            QT = [persist.tile([128, L], BF16, tag=f"qt{h}", name=f"qt{h}") for h in range(QH)]
            KT = [persist.tile([128, L], BF16, tag=f"kt{g}", name=f"kt{g}") for g in range(KVH)]
            # V extended with a ones column per k-tile: [128, 16*129]
            VE = [persist.tile([128, LT * 129], BF16, tag=f"ve{g}", name=f"ve{g}") for g in range(KVH)]

            ident = const.tile([128, 128], BF16)
            cost = const.tile([128, L], BF16)
            sint = const.tile([128, L], BF16)
            maskt = const.tile([128, 128], BF16)
            wvs = wvp.tile([128, KC * 256], BF16, name="wvs")

            deferred = _phase1(nc, tc, ident, cost, sint, maskt, wvs, QT, KT, VE,
                               xq_d, wqq_d, wkh_d, wvh_d, cost_d, sint_d,
                               masks_d, xpb)
            _phase2(nc, tc, ident, maskt, QT, KT, VE, wvs, wo4_d, out_d, deferred)
    return nc


def _phase1(nc, tc, ident, cost, sint, maskt, wvs, QT, KT, VE,
            xq_d, wqq_d, wkh_d, wvh_d, cost_d, sint_d, masks_d, xpb):
    with (
        tc.tile_pool(name="xpa", bufs=1) as xpa,
        tc.tile_pool(name="wqk", bufs=1) as wqk,
        tc.tile_pool(name="rtmp", bufs=3) as rtmp,
        tc.tile_pool(name="qk_psum", bufs=1, space="PSUM") as qk_psum,
        tc.tile_pool(name="vp_psum", bufs=2, space="PSUM") as vp_psum,
    ):
        wqs = wqk.tile([128, KC * 512], BF16, name="wqs")
        wks = wqk.tile([128, KC * 256], BF16, name="wks")

        # startup: k=0 slivers first so the first matmuls start ~3us in;
        # the rest lands in k-ascending order, always ahead of the k-outer
        # consumption (one 512-col x slice feeds 6 N=512 matmuls = 1.3us,
        # vs ~0.4us to transfer).
        x_first = xpa.tile([128, KC * 512], BF16, tag="xa", name="x0")
        nc.sync.dma_start(wqs[:, 0:512], wqq_d[0][:, 0:512])
        nc.sync.dma_start(x_first[:, 0:512], xq_d[0, 0][:, 0:512])
        nc.sync.dma_start(wks[:, 0:256], wkh_d[0][:, 0:256])
        nc.sync.dma_start(wqs[:, 512:1536], wqq_d[0][:, 512:1536])
        nc.sync.dma_start(x_first[:, 512:1536], xq_d[0, 0][:, 512:1536])
        nc.sync.dma_start(wks[:, 256:768], wkh_d[0][:, 256:768])
        nc.sync.dma_start(wqs[:, 1536:2560], wqq_d[0][:, 1536:2560])
        nc.sync.dma_start(x_first[:, 1536:2560], xq_d[0, 0][:, 1536:2560])
        nc.sync.dma_start(wqs[:, 2560:QUART], wqq_d[0][:, 2560:QUART])
        nc.sync.dma_start(x_first[:, 2560:QUART], xq_d[0, 0][:, 2560:QUART])
        nc.sync.dma_start(wks[:, 768:18 * 256], wkh_d[0][:, 768:18 * 256])
        nc.sync.dma_start(wqs[:, QUART:QUART + 2048], wqq_d[1][:, 0:2048])
        nc.sync.dma_start(x_first[:, QUART:QUART + 2048], xq_d[0, 1][:, 0:2048])
        nc.sync.dma_start(wqs[:, QUART + 2048:2 * QUART], wqq_d[1][:, 2048:QUART])
        nc.sync.dma_start(x_first[:, QUART + 2048:2 * QUART], xq_d[0, 1][:, 2048:QUART])
        for j in range(2, 4):
            nc.sync.dma_start(wqs[:, j * QUART:(j + 1) * QUART], wqq_d[j])
            nc.sync.dma_start(x_first[:, j * QUART:(j + 1) * QUART], xq_d[0, j])
        nc.sync.dma_start(wks[:, 18 * 256:36 * 256], wkh_d[1])
        nc.sync.dma_start(cost[:], cost_d[:])
        nc.sync.dma_start(sint[:], sint_d[:])
        for j in range(2):
            nc.sync.dma_start(wvs[:, j * 18 * 256:(j + 1) * 18 * 256], wvh_d[j])
        nc.sync.dma_start(maskt[:], masks_d[:])
        make_identity(nc, ident[:])

        def load_x(nq, dst):
            for j in range(4):
                nc.sync.dma_start(dst[:, j * QUART:(j + 1) * QUART], xq_d[nq, j])

        def drain_rope(ps, dst, nq, staged=False):
            """psum [128,512] f32 -> rope -> dst bf16 [128,512] slice.
            staged=True reads the psum once through a DVE copy so the bank
            (and at chunk 3, the whole phase-1 PSUM pool stack) releases
            after ~0.55us instead of after the 3-op read chain."""
            cols = slice(nq * 512, (nq + 1) * 512)
            if staged:
                stage = rtmp.tile([128, 512], F32, tag="stage")
                nc.vector.tensor_copy(stage[:], ps[:])
                ps = stage
            raw = rtmp.tile([128, 512], F32, tag="raw")
            nc.vector.tensor_mul(raw[:], ps[:], cost[:, cols])
            swap = rtmp.tile([128, 512], F32, tag="swap")
            nc.scalar.activation(swap[0:64, :], ps[64:128, :], AF.Copy)
            nc.scalar.activation(swap[64:128, :], ps[0:64, :], AF.Copy)
            nc.gpsimd.tensor_mul(swap[:], swap[:], sint[:, cols])
            nc.gpsimd.tensor_add(dst[:, cols], raw[:], swap[:])

        def alloc_x(nq):
            pool = xpb if nq % 2 == 1 else xpa
            tag = "xb" if nq % 2 == 1 else "xa"
            return pool.tile([128, KC * 512], BF16, tag=tag, name=f"x{nq}")

        xc = x_first
        for nq in range(NQ):
            if nq + 1 < NQ:
                xnext = alloc_x(nq + 1)
                load_x(nq + 1, xnext)
            if nq == 0:
                # k-outer: consumes x/wq strictly in k (DMA) order so the
                # startup never outruns the loads; drains bunch at the end
                # but overlap chunk 1's compute.
                psq = [qk_psum.tile([128, 512], F32, tag=f"q{h}", name=f"psq{h}")
                       for h in range(QH)]
                psk = [qk_psum.tile([128, 512], F32, tag=f"k{g}", name=f"psk{g}")
                       for g in range(KVH)]
                for k in range(KC):
                    for h in range(QH):
                        nc.tensor.matmul(
                            psq[h][:], wqs[:, k * 512 + h * 128:k * 512 + (h + 1) * 128],
                            xc[:, k * 512:(k + 1) * 512],
                            start=(k == 0), stop=(k == KC - 1))
                    for g in range(KVH):
                        nc.tensor.matmul(
                            psk[g][:], wks[:, k * 256 + g * 128:k * 256 + (g + 1) * 128],
                            xc[:, k * 512:(k + 1) * 512],
                            start=(k == 0), stop=(k == KC - 1))
                for h in range(QH):
                    drain_rope(psq[h], QT[h], nq)
                for g in range(KVH):
                    drain_rope(psk[g], KT[g], nq)
            else:
                # h-outer: x is fully prefetched by now; per-head psums drain
                # staggered so the engine queues (and the phase-2 handover for
                # chunk 3) never sit behind six bunched rope chains.
                for h in range(QH):
                    ps = qk_psum.tile([128, 512], F32, tag=f"q{h}", name=f"psq{h}")
                    for k in range(KC):
                        nc.tensor.matmul(
                            ps[:], wqs[:, k * 512 + h * 128:k * 512 + (h + 1) * 128],
                            xc[:, k * 512:(k + 1) * 512],
                            start=(k == 0), stop=(k == KC - 1))
                    drain_rope(ps, QT[h], nq, staged=(nq == NQ - 1))
                for g in range(KVH):
                    ps = qk_psum.tile([128, 512], F32, tag=f"k{g}", name=f"psk{g}")
                    for k in range(KC):
                        nc.tensor.matmul(
                            ps[:], wks[:, k * 256 + g * 128:k * 256 + (g + 1) * 128],
                            xc[:, k * 512:(k + 1) * 512],
                            start=(k == 0), stop=(k == KC - 1))
                    drain_rope(ps, KT[g], nq)
            if nq == NQ - 1:
                return [(nq, sub, xc) for sub in range(4)]  # V deferred to phase 2
            # V sweep of this chunk; its PE work covers the rope drains above
            for sub in range(4):
                mk = nq * 4 + sub
                ps = vp_psum.tile([128, KVH * 128], F32, tag="vps")
                for k in range(KC):
                    nc.tensor.matmul(
                        ps[:], xc[:, k * 512 + sub * 128:k * 512 + (sub + 1) * 128],
                        wvs[:, k * 256:(k + 1) * 256],
                        start=(k == 0), stop=(k == KC - 1))
                for g in range(KVH):
                    nc.vector.tensor_copy(
                        VE[g][:, mk * 129:mk * 129 + 128],
                        ps[:, g * 128:(g + 1) * 128])
                    nc.gpsimd.memset(VE[g][:, mk * 129 + 128:mk * 129 + 129], 1.0)
            xc = xnext


def _phase2(nc, tc, ident, maskt, QT, KT, VE, wvs, wo4_d, out_d, deferred_v):
    with (
        tc.tile_pool(name="wo", bufs=1) as wop,
        tc.tile_pool(name="pt", bufs=3) as ptp,
        tc.tile_pool(name="tt", bufs=2) as ttp,
        tc.tile_pool(name="attnt", bufs=2) as attp,
        tc.tile_pool(name="small", bufs=2) as small,
        tc.tile_pool(name="ostage", bufs=2) as ostage,
        tc.tile_pool(name="sc_psum", bufs=2, space="PSUM") as sc_psum,
        tc.tile_pool(name="pv_psum", bufs=1, space="PSUM") as pv_psum,
        tc.tile_pool(name="op_psum", bufs=2, space="PSUM") as op_psum,
        tc.tile_pool(name="atr_psum", bufs=1, space="PSUM") as atr_psum,
    ):
        WO = wop.tile([128, QH * D], BF16, name="wos")
        for h in range(QH):
            nc.sync.dma_start(WO[:, h * D:(h + 1) * D], wo4_d[h])

        # Wave order [1, 2, 0, 3]: each wave's softcap/exp latency is covered
        # by exp-independent PE filler - wave 1 by the deferred-V units (plus
        # its own larger score matmuls), wave 2 by fresh o_proj(1), thin wave
        # 0 by o_proj(2) whose surplus carries into wave 3 alongside
        # o_proj(0); the tail is o_proj(3), which is dense PE work. nq-major
        # order instead starved waves 2-3 (filler exhausted early) and
        # re-throttled the HAM clock gate.
        groups = [(nq, h) for nq in (1, 2, 0, 3) for h in range(QH)]
        state = {}
        att_of = {nq: [None] * QH for nq in range(NQ)}

        # Two filler queues:
        #   qA: exp-dependent units (PV / transpose), entries
        #       (elig_pair, group, pe_cost_ns, emit_fn) - eligible once the
        #       global score-pair counter passes 2 pairs beyond their group.
        #   qB: exp-independent units (o_proj, deferred V), entries
        #       (group, pe_cost_ns, emit_fn) - eligible once qA holds nothing
        #       at or before their group (preserves the T(nq,3) -> o_proj(nq)
        #       emission-order dependency).
        qA = deque()
        qB = deque()
        held_back = []            # final-flush filler (see end of function)
        pairs_done = [0]
        prefer_b = [False]
        queue_cost = [0]          # summed pe_cost_ns of everything queued

        def emit_next(i):
            """Emit one eligible filler unit; return its PE cost or None.
            The two queues alternate so a PV unit's psum-recycle (DVE
            recip+mul) never sits behind several o_proj drain casts in the
            DVE queue. qB entries with group None (deferred V) have no
            ordering dependency on qA at all."""
            a_ok = qA and qA[0][0] <= pairs_done[0]
            b_ok = qB and (qB[0][0] is None or not qA or qA[0][1] > qB[0][0])
            use_b = b_ok and (prefer_b[0] or not a_ok)
            if use_b:
                g, cost, fn = qB.popleft()
                prefer_b[0] = False
            elif a_ok:
                e, g, cost, fn = qA.popleft()
                prefer_b[0] = True
            else:
                return None
            fn()
            queue_cost[0] -= cost
            return cost

        def pump(i, pairs_left_in_wave):
            # spread the queued filler across the rest of this wave's score
            # pairs (+2 reserves a couple of units for the final flush): a
            # fixed 2200ns/pair drained each wave's o_proj supply before its
            # last groups, leaving them exp-gated in 1-3us clustered gaps
            # that re-throttle the HAM clock gate; spreading converts those
            # into harmless sub-window micro-gaps.
            ns = min(3200, max(900, queue_cost[0] // (pairs_left_in_wave + 2)))
            while ns > 0:
                c = emit_next(i)
                if c is None:
                    return
                ns -= c

        def pump_guard(i):
            # bound pipeline lag: PV/tr of group <= i-3 must be emitted
            # before group i reuses their pt buffer (pt pool is
            # triple-buffered). o_proj of wave W only conflicts with wave
            # W+2's attnT writes (attnt pool is double-buffered), so its
            # bound is i-5 - flushing it at i-3 front-loaded each wave's
            # filler and starved the next wave's tail. Deferred V units
            # (group None) are only forced once wave 3 approaches needing
            # VE; otherwise they interleave into wave 1's exp-latency
            # windows via pump().
            while qA and qA[0][1] <= i - 3:
                qA.popleft()[3]()
            while qB and ((qB[0][0] is None and i >= 8)
                          or (qB[0][0] is not None and qB[0][0] <= i - 5)):
                qB.popleft()[2]()

        def make_v_unit(vnq, sub, xc):
            def emit():
                mk = vnq * 4 + sub
                ps = op_psum.tile([128, 512], F32, tag="op")
                for k in range(KC):
                    nc.tensor.matmul(
                        ps[:, 0:KVH * 128],
                        xc[:, k * 512 + sub * 128:k * 512 + (sub + 1) * 128],
                        wvs[:, k * 256:(k + 1) * 256],
                        start=(k == 0), stop=(k == KC - 1))
                for g in range(KVH):
                    nc.vector.tensor_copy(
                        VE[g][:, mk * 129:mk * 129 + 128],
                        ps[:, g * 128:(g + 1) * 128])
                    nc.gpsimd.memset(VE[g][:, mk * 129 + 128:mk * 129 + 129], 1.0)
            return (None, 3900, emit)

        for u in deferred_v:
            unit = make_v_unit(*u)
            queue_cost[0] += unit[1]
            qB.append(unit)

        # bridge the phase-boundary handover with one deferred-V unit: the
        # first score pair's tt/pt tiles sit in fresh SBUF pools that wait on
        # phase-1's rtmp close (the ~2.4us chunk-3 rope-drain chain), but a
        # V unit needs only PSUM banks and persistent tiles, so its 36
        # matmuls keep the PE warm across the gap.
        g, cost, fn = qB.popleft()
        queue_cost[0] -= cost
        fn()

        def emit_scores_pair(nq, h, pk, pt):
            """two score tiles into one [128,1024] f32 psum pair -> a single
            trimmed tanh into fp16 staging -> exp immediately (per-pair).
            Any stale psum between the two trimmed halves lands in pt columns
            that PV provably never reads."""
            g = h // 2
            ps = sc_psum.tile([128, 1024], F32, tag="sc")
            c0s = []
            for half in range(2):
                mk = 2 * pk + half
                o = mk - 4 * nq
                c0 = max(0, o) * 128
                c0s.append(c0)
                nc.tensor.matmul(
                    ps[:, half * 512 + c0:half * 512 + 512],
                    KT[g][:, mk * 128:(mk + 1) * 128],
                    QT[h][:, nq * 512 + c0:(nq + 1) * 512])
            tt = ttp.tile([128, 1024], FP16, tag="tt")
            nc.scalar.activation(
                tt[:, c0s[0]:1024], ps[:, c0s[0]:1024],
                AF.Tanh, scale=SCALE / SOFTCAP)
            nc.scalar.activation(
                pt[:, pk * 1024 + c0s[0]:(pk + 1) * 1024], tt[:, c0s[0]:1024],
                AF.Exp, scale=SOFTCAP)

        def emit_masks(nq, h, pt):
            # only the 128x128 block on the causal diagonal of each diagonal
            # tile is partially masked; everything left of it is never read
            for o in range(4):
                mk = 4 * nq + o
                base = mk * 512 + o * 128
                nc.gpsimd.tensor_mul(
                    pt[:, base:base + 128], pt[:, base:base + 128], maskt[:])

        def make_pv_unit(nq, h, s):
            def emit():
                st = state[(nq, h)]
                g = h // 2
                nks = 4 * nq + s + 1
                pv = pv_psum.tile([128, 129], F32, tag="pv")
                for mk in range(nks):
                    nc.tensor.matmul(
                        pv[:],
                        st["pt"][:, mk * 512 + s * 128:mk * 512 + (s + 1) * 128],
                        VE[g][:, mk * 129:(mk + 1) * 129],
                        start=(mk == 0), stop=(mk == nks - 1))
                recip = small.tile([128, 1], F32, tag="recip")
                nc.vector.reciprocal(recip[:], pv[:, 128:129])
                aq = small.tile([128, 128], BF16, tag=f"attnq{s}")
                nc.vector.tensor_scalar_mul(aq[:], pv[:, 0:128], recip[:])
                st["attnq"][s] = aq
            return 200 + (4 * nq + s + 1) * 110, emit

        def make_tr_unit(nq, h, s):
            def emit():
                st = state[(nq, h)]
                tp = atr_psum.tile([128, 128], BF16, tag="atr")
                nc.tensor.transpose(tp[:], st["attnq"][s][:], ident[:])
                nc.vector.tensor_copy(st["attnT"][:, s * 128:(s + 1) * 128], tp[:])
            return 180, emit

        def make_oproj_unit(idx, nq, s, j, ob, split_store=False):
            """single 512-wide o_proj chunk; DMA fires on each 3-chunk group
            boundary so at most one DVE drain sits between pumped PV units.
            The very last row stores per-512 so the final transfer is small.
            Units that only ever run in the tail (the last wave's, and the
            held-back final-flush cover) drain on the scalar engine - idle
            there, and it reads PSUM faster than DVE does."""
            tail_time = nq == 3 or (idx == len(groups) - 5 and s == 3 and j >= 3)
            def emit():
                po = op_psum.tile([128, 512], F32, tag="op")
                for h in range(QH):
                    nc.tensor.matmul(
                        po[:], att_of[nq][h][:, s * 128:(s + 1) * 128],
                        WO[:, h * D + j * 512:h * D + (j + 1) * 512],
                        start=(h == 0), stop=(h == QH - 1))
                dst = ob[:, (j % 3) * 512:(j % 3 + 1) * 512]
                if tail_time:
                    nc.scalar.activation(dst, po[:], AF.Copy)
                else:
                    nc.vector.tensor_copy(dst, po[:])
                row = nq * 512 + s * 128
                if split_store:
                    nc.sync.dma_start(
                        out_d[row:row + 128, j * 512:(j + 1) * 512],
                        ob[:, (j % 3) * 512:(j % 3 + 1) * 512])
                elif j % 3 == 2:
                    jg = j // 3
                    nc.sync.dma_start(
                        out_d[row:row + 128, jg * 1536:(jg + 1) * 1536], ob[:])
            return (idx, 1050, emit)

        for i, (nq, h) in enumerate(groups):
            npairs = 2 * nq + 2
            attnT = attp.tile([128, 512], BF16, tag=f"at{h}", name=f"at{h}")
            att_of[nq][h] = attnT
            pt = ptp.tile([128, LT * 512], BF16, tag="pt", name="pt")
            st = {"pt": pt, "attnq": [None] * 4, "attnT": attnT}
            state[(nq, h)] = st
            for pk in range(npairs):
                pump_guard(i)
                emit_scores_pair(nq, h, pk, pt)
                pairs_done[0] += 1
                pump(i, (4 - i % 4) * npairs - pk - 1)
            emit_masks(nq, h, pt)
            for s in range(4):
                # PV(s) only reads score tiles mk <= 4nq+s, so s=0,1 gate on
                # the second-to-last pair's exp and become eligible a pair
                # earlier than s=2,3. Wave-1 groups (i<4) wait one pair
                # longer: their filler (deferred V) bypasses the qA ordering
                # check, so late eligibility costs nothing, and emitting PV
                # before its exp cleared was measured as 0.7-1us PE waits.
                elig = pairs_done[0] + (1 if s < 2 else 2) + (1 if i < 4 else 0)
                cost, fn = make_pv_unit(nq, h, s)
                qA.append((elig, i, cost, fn))
                queue_cost[0] += cost
                cost, fn = make_tr_unit(nq, h, s)
                qA.append((elig, i, cost, fn))
                queue_cost[0] += cost
            if h == QH - 1:
                last_row = i == len(groups) - 1
                for s in range(4):
                    split = last_row and s == 3
                    ob = None
                    for j in range(D // 512):
                        if j % 3 == 0:
                            ob = ostage.tile([128, 1536], BF16, tag="ob")
                        unit = make_oproj_unit(i, nq, s, j, ob, split)
                        if i == len(groups) - 5 and s == 3 and j >= 3:
                            held_back.append(unit)  # final-flush exp cover
                        else:
                            queue_cost[0] += unit[1]
                            qB.append(unit)
        # cover the last group's exp latency with the held-back o_proj
        # units before its PV units hit the PE queue; they are from an
        # earlier wave, so their attnT reads are long satisfied, and they
        # emit before the last wave's o_proj so the ostage write-after-read
        # chain stays in order
        for g, cost, fn in held_back:
            fn()
        while qA:
            qA.popleft()[3]()
        while qB:
            qB.popleft()[2]()


_CACHED_NC = {}


def build():
    if "nc" not in _CACHED_NC:
        nc = bacc.Bacc("TRN2", target_bir_lowering=False, debug=False)
        _emit(nc)
        nc.compile()
        _CACHED_NC["nc"] = nc
    return _CACHED_NC["nc"]


def host_tables():
    inv_freq = 1.0 / (ROPE_THETA ** (np.arange(0, HEAD_DIM, 2, dtype=np.float32) / HEAD_DIM))
    ang = np.arange(L, dtype=np.float32)[:, None] * inv_freq[None, :]  # [L, 64]
    cos, sin = np.cos(ang), np.sin(ang)
    cosT = np.concatenate([cos.T, cos.T], axis=0).astype(BF16_NP)
    sinT = np.concatenate([-sin.T, sin.T], axis=0).astype(BF16_NP)
    return np.ascontiguousarray(cosT), np.ascontiguousarray(sinT)


def host_masks():
    k = np.arange(128)[:, None]
    q = np.arange(128)[None, :]
    return np.ascontiguousarray((q >= k).astype(BF16_NP))  # [128, 128] triangle


def make_in_maps(x, wq, wk, wv, wo):
    cosT, sinT = host_tables()
    masks = host_masks()
    xt = x.reshape(L, D).T.astype(BF16_NP)  # [D, L]
    # x image: [nq, j, p, k9*512+c] = xt[(9j+k9)*128+p, nq*512+c]
    xi = xt.reshape(KC, 128, NQ, 512).transpose(2, 0, 1, 3)  # [NQ, KC, 128, 512]
    xi = xi.reshape(NQ, 4, 9, 128, 512).transpose(0, 1, 3, 2, 4)
    xi = np.ascontiguousarray(xi.reshape(NQ, 4, 128, QUART))
    in_maps = []
    for c in range(N_CORES):
        qs = slice(c * QH * 128, (c + 1) * QH * 128)
        kvs = slice(c * KVH * 128, (c + 1) * KVH * 128)
        wqt = wq[qs].T.astype(BF16_NP)   # [D, 512]
        wkt = wk[kvs].T.astype(BF16_NP)  # [D, 256]
        wvt = wv[kvs].T.astype(BF16_NP)
        wot = wo[:, qs].T.astype(BF16_NP)  # [512, D]
        wqi = wqt.reshape(KC, 128, 512).transpose(1, 0, 2)
        wqi = np.ascontiguousarray(
            wqi.reshape(128, 4, 9, 512).transpose(1, 0, 2, 3).reshape(4, 128, QUART))
        wki = wkt.reshape(KC, 128, 256).transpose(1, 0, 2)
        wki = np.ascontiguousarray(
            wki.reshape(128, 2, 18, 256).transpose(1, 0, 2, 3).reshape(2, 128, 18 * 256))
        wvi = wvt.reshape(KC, 128, 256).transpose(1, 0, 2)
        wvi = np.ascontiguousarray(
            wvi.reshape(128, 2, 18, 256).transpose(1, 0, 2, 3).reshape(2, 128, 18 * 256))
        wo4 = np.ascontiguousarray(wot.reshape(QH, 128, D))
        in_maps.append({
            "xq": xi,
            "wqq": wqi,
            "wkh": wki,
            "wvh": wvi,
            "wo4": wo4,
            "cost": cosT,
            "sint": sinT,
            "masks": masks,
        })
    return in_maps


def run(inputs, trace=False, trace_kwargs=None):
    from concourse.bass_utils import run_bass_kernel_spmd

    nc = build()
    x = np.asarray(inputs["x"], dtype=np.float32)
    in_maps = make_in_maps(
        x,
        np.asarray(inputs["wq"], dtype=np.float32),
        np.asarray(inputs["wk"], dtype=np.float32),
        np.asarray(inputs["wv"], dtype=np.float32),
        np.asarray(inputs["wo"], dtype=np.float32),
    )
    res = run_bass_kernel_spmd(
        nc, in_maps, core_ids=list(range(N_CORES)),
        trace=trace, **(trace_kwargs or {}))
    out = np.zeros((L, D), dtype=np.float32)
    for c in range(N_CORES):
        out += res.results[c]["out"].astype(np.float32)
    return out.reshape(x.shape), res


def kernel(**inputs) -> np.ndarray:
    out, _ = run(inputs, trace=False)
    return out
